# revision 1
# baseline (speedup 1.0000x reference)
"""Distributed GQA attention kernel for one TRN2 chip (8 NeuronCores).

Problem: B=2, L=2048, HID=2048, H=32 q-heads, HKV=8 kv-heads, D=64,
rotary embedding, causal softmax, o-proj.

Sharding: core i -> batch b=i//4, TP rank r=i%4.  Each core computes
8 q-heads / 2 kv-heads of its batch, all-gathers the attention outputs
(feature-major, bf16) within its 4-core TP group, then computes its
512 output columns of the o-proj.  Host assembles the full output.

All matmuls run in bf16 with fp32 PSUM accumulation.  Softmax skips the
row-max (logits are bounded ~|6| for these input scales) and obtains
row sums for free by appending a 64-wide ones block to V's stationary
operand; normalization is a DVE reciprocal + multiply.
"""

import sys

sys.path.insert(0, "/opt/trn_rl_repo")

import numpy as np
import ml_dtypes

B, L, HID = 2, 2048, 2048
H, HKV, D = 32, 8, 64
N_CORES = 8
TP = 4           # tensor-parallel group size
HL = 8           # q heads per core
CW = 512         # o-proj output columns per core
TT = 4           # t tiles of 512 over L
CCH = HID // 128 # contraction chunks (16)
BF16 = ml_dtypes.bfloat16

_cache = {}


def _build_graph(dbg=None):
    import concourse.bass as bass
    import concourse.tile as tile
    from concourse import bacc, mybir

    dt = mybir.dt
    f32, bf16 = dt.float32, dt.bfloat16

    nc = bacc.Bacc("TRN2", target_bir_lowering=False, debug=False,
                   num_devices=N_CORES)

    xT = nc.dram_tensor("xT", [HID, L], bf16, kind="ExternalInput")
    WqT = nc.dram_tensor("WqT", [HID, HL * D], bf16, kind="ExternalInput")
    WkT = nc.dram_tensor("WkT", [HID, 128], bf16, kind="ExternalInput")
    WvT = nc.dram_tensor("WvT", [HID, 128], bf16, kind="ExternalInput")
    WoT = nc.dram_tensor("WoT", [HID, CW], bf16, kind="ExternalInput")
    C1q = nc.dram_tensor("C1q", [128, L], bf16, kind="ExternalInput")
    C2q = nc.dram_tensor("C2q", [128, L], bf16, kind="ExternalInput")
    C1k = nc.dram_tensor("C1k", [128, L], bf16, kind="ExternalInput")
    C2k = nc.dram_tensor("C2k", [128, L], bf16, kind="ExternalInput")
    out = nc.dram_tensor("out", [CW, L], f32, kind="ExternalOutput")
    dbg_shapes = {"qq": [128, HL // 2 * L], "kk": [128, L],
                  "v2": [128, CCH * 256], "ao": [128, HL // 2 * L],
                  "gathered": [TP * TP * 128, L], "bounce": [TP * 128, L]}
    dbg_t = (nc.dram_tensor("dbg", dbg_shapes[dbg], bf16,
                            kind="ExternalOutput") if dbg else None)

    def bcast_m(ap2d, n):
        # [P, F] -> [P, n, F] with a step-0 middle dim (free-dim broadcast)
        return bass.AP(ap2d.tensor, ap2d.offset,
                       [ap2d.ap[0], [0, n], ap2d.ap[1]])

    with tile.TileContext(nc) as tc:
        with (
            tc.tile_pool(name="persist", bufs=1) as persist,
            tc.tile_pool(name="ps", bufs=2, space="PSUM") as ps,
            tc.tile_pool(name="psbig", bufs=2, space="PSUM") as psbig,
            tc.tile_pool(name="pp", bufs=3) as pp,
            tc.tile_pool(name="dram", bufs=1, space="DRAM") as dram,
        ):
            # ---- persistent SBUF tensors ----
            qq = persist.tile([128, HL // 2 * L], bf16)      # roped Q^T, 2MB
            kk = persist.tile([128, L], bf16)                # roped K^T (2 kv)
            v2t = persist.tile([128, L], bf16)               # V^T staging
            v2 = persist.tile([128, CCH * 256], bf16)        # [V|1|V|1] per kt
            ao = persist.tile([128, HL // 2 * L], bf16)      # attn out^T
            c1q = persist.tile([128, L], bf16)
            c2q = persist.tile([128, L], bf16)
            c1k = persist.tile([128, L], bf16)
            c2k = persist.tile([128, L], bf16)
            for t_sb, t_dr in ((c1q, C1q), (c2q, C2q), (c1k, C1k), (c2k, C2k)):
                nc.scalar.dma_start(t_sb[:], t_dr[:])

            # ones blocks of v2 (columns 64:128 and 192:256 of each kt group)
            for off in (64, 192):
                ones_view = bass.AP(v2.tensor, v2.offset + off,
                                    [v2.ap[0], [256, CCH], [1, 64]])
                nc.gpsimd.memset(ones_view, 1.0)

            # =========== Phase B: projections + RoPE ===========
            with (
                tc.tile_pool(name="wsb", bufs=1) as wsb,
                tc.tile_pool(name="xt", bufs=2) as xtp,
                tc.tile_pool(name="rope", bufs=2) as rope,
            ):
                wq_sb = wsb.tile([128, CCH * 512], bf16)
                wq_v = WqT[:].rearrange("(c p) m -> p c m", p=128)
                wq_s = wq_sb[:].rearrange("p (c m) -> p c m", m=512)
                for cb in range(4):
                    nc.scalar.dma_start(wq_s[:, 4 * cb:4 * (cb + 1)],
                                        wq_v[:, 4 * cb:4 * (cb + 1)])
                wk_sb = wsb.tile([128, CCH * 128], bf16)
                nc.scalar.dma_start(
                    wk_sb[:].rearrange("p (c m) -> p c m", m=128),
                    WkT[:].rearrange("(c p) m -> p c m", p=128))
                wv_sb = wsb.tile([128, CCH * 128], bf16)
                nc.scalar.dma_start(
                    wv_sb[:].rearrange("p (c m) -> p c m", m=128),
                    WvT[:].rearrange("(c p) m -> p c m", p=128))

                xT_view = xT[:].rearrange("(c p) t -> p c t", p=128)

                for tt in range(TT):
                    ts = slice(tt * 512, (tt + 1) * 512)
                    xt = xtp.tile([128, CCH * 512], bf16, tag="xt")
                    xt_s = xt[:].rearrange("p (c t) -> p c t", t=512)
                    for cb in range(4):
                        nc.sync.dma_start(xt_s[:, 4 * cb:4 * (cb + 1)],
                                          xT_view[:, 4 * cb:4 * (cb + 1), ts])

                    # --- Q: 4 M-tiles (head pair (jj, jj+4) each) ---
                    qraw = rope.tile([128, 4 * 512], bf16, tag="qraw")
                    for m in range(4):
                        psq = ps.tile([128, 512], f32, tag="mm")
                        for c in range(CCH):
                            nc.tensor.matmul(
                                psq[:],
                                lhsT=wq_sb[:, c * 512 + m * 128:
                                           c * 512 + (m + 1) * 128],
                                rhs=xt[:, c * 512:(c + 1) * 512],
                                start=(c == 0), stop=(c == CCH - 1))
                        nc.scalar.copy(qraw[:, m * 512:(m + 1) * 512], psq[:])

                    # --- K ---
                    kraw = rope.tile([128, 512], bf16, tag="kraw")
                    psk = ps.tile([128, 512], f32, tag="mm")
                    for c in range(CCH):
                        nc.tensor.matmul(
                            psk[:], lhsT=wk_sb[:, c * 128:(c + 1) * 128],
                            rhs=xt[:, c * 512:(c + 1) * 512],
                            start=(c == 0), stop=(c == CCH - 1))
                    nc.scalar.copy(kraw[:], psk[:])

                    # --- V ---
                    psv = ps.tile([128, 512], f32, tag="mm")
                    for c in range(CCH):
                        nc.tensor.matmul(
                            psv[:], lhsT=wv_sb[:, c * 128:(c + 1) * 128],
                            rhs=xt[:, c * 512:(c + 1) * 512],
                            start=(c == 0), stop=(c == CCH - 1))
                    nc.scalar.copy(v2t[:, ts], psv[:])

                    # --- RoPE on Q ---
                    qsw = rope.tile([128, 4 * 512], bf16, tag="qsw")
                    for a, b_ in ((0, 32), (32, 0), (64, 96), (96, 64)):
                        nc.scalar.dma_start(qsw[b_:b_ + 32, :], qraw[a:a + 32, :])
                    qtmp = rope.tile([128, 4 * 512], bf16, tag="qtmp")
                    q3 = qraw[:].rearrange("p (m t) -> p m t", t=512)
                    s3 = qsw[:].rearrange("p (m t) -> p m t", t=512)
                    t3 = qtmp[:].rearrange("p (m t) -> p m t", t=512)
                    nc.vector.tensor_tensor(t3, q3, bcast_m(c1q[:, ts], 4),
                                            mybir.AluOpType.mult)
                    nc.vector.tensor_tensor(s3, s3, bcast_m(c2q[:, ts], 4),
                                            mybir.AluOpType.mult)
                    qqd = bass.AP(qq.tensor, qq.offset + tt * 512,
                                  [qq.ap[0], [2048, 4], [1, 512]])
                    nc.vector.tensor_tensor(qqd, t3, s3, mybir.AluOpType.add)

                    # --- RoPE on K ---
                    ksw = rope.tile([128, 512], bf16, tag="ksw")
                    for a, b_ in ((0, 32), (32, 0), (64, 96), (96, 64)):
                        nc.scalar.dma_start(ksw[b_:b_ + 32, :], kraw[a:a + 32, :])
                    ktmp = rope.tile([128, 512], bf16, tag="ktmp")
                    nc.vector.tensor_tensor(ktmp[:], kraw[:], c1k[:, ts],
                                            mybir.AluOpType.mult)
                    nc.vector.tensor_tensor(ksw[:], ksw[:], c2k[:, ts],
                                            mybir.AluOpType.mult)
                    nc.vector.tensor_tensor(kk[:, ts], ktmp[:], ksw[:],
                                            mybir.AluOpType.add)

                    # --- V transpose to token-major (v2 cols 0:64/128:192),
                    # per-tt so attention can start before proj finishes
                    for g in range(2):
                        v2_dst = bass.AP(v2.tensor,
                                         v2.offset + (4 * tt) * 256 + g * 128,
                                         [v2.ap[0], [256, 4], [1, 64]])
                        nc.sync.dma_start_transpose(
                            v2_dst, v2t[g * 64:(g + 1) * 64, ts])

            # =========== Phases C+D: attention with interleaved AG/o-proj ====
            # Causal mask is folded into the S-matmul PSUM accumulation via an
            # extra matmul: ident.T @ Bmask_j adds -48 where q_global<k_global,
            # so exp() gives ~e^-45 (negligible) with no post-exp masking.
            ident = persist.tile([128, 128], bf16)
            nc.gpsimd.memset(ident[:], 1.0)
            nc.gpsimd.affine_select(
                out=ident[:], in_=ident[:], pattern=[[-1, 128]],
                compare_op=mybir.AluOpType.is_equal, fill=0.0,
                base=0, channel_multiplier=1)
            bmask = persist.tile([128, 4 * 512], bf16)
            nc.gpsimd.memset(bmask[:], -48.0)
            for j in range(4):
                # keep -48 where q < 128j + k, else 0
                nc.gpsimd.affine_select(
                    out=bmask[:, j * 512:(j + 1) * 512],
                    in_=bmask[:, j * 512:(j + 1) * 512],
                    pattern=[[-1, 512]], compare_op=mybir.AluOpType.is_gt,
                    fill=0.0, base=128 * j, channel_multiplier=1)

            bounces = [dram.tile([2 * 128, L], bf16, name=f"bounce{h}")
                       for h in range(2)]
            gath = [dram.tile([TP * 2 * 128, L], bf16, name=f"gath{h}")
                    for h in range(2)]

            with (
                tc.tile_pool(name="wo", bufs=1) as wop,
                tc.tile_pool(name="aog", bufs=36) as aogp,
                tc.tile_pool(name="sta", bufs=16) as stap,
                tc.tile_pool(name="ost", bufs=2) as ostp,
            ):
                wo_sb = wop.tile([128, CCH * 512], bf16)
                nc.scalar.dma_start(
                    wo_sb[:].rearrange("p (c m) -> p c m", m=512),
                    WoT[:].rearrange("(c p) m -> p c m", p=128))

                def attention_half(hh):
                    for jj in (2 * hh, 2 * hh + 1):
                        qoff = jj * L
                        for qT in range(4):
                            nkt = 4 * qT + 4
                            qs = slice(qoff + qT * 512, qoff + (qT + 1) * 512)
                            o0 = ps.tile([128, 512], f32, tag="o",
                                         name=f"o0_{jj}_{qT}")
                            o1 = ps.tile([128, 512], f32, tag="o",
                                         name=f"o1_{jj}_{qT}")
                            for kp in range(nkt // 2):
                                sb0 = psbig.tile([128, 1024], f32, tag="s",
                                                 name=f"sb0_{jj}_{qT}_{kp}")
                                sb1 = psbig.tile([128, 1024], f32, tag="s",
                                                 name=f"sb1_{jj}_{qT}_{kp}")
                                for h in range(2):
                                    kt = 2 * kp + h
                                    ksl = slice(kt * 128, (kt + 1) * 128)
                                    hs = slice(h * 512, (h + 1) * 512)
                                    dj = kt - 4 * qT  # >=0 -> diagonal block
                                    nc.tensor.matmul(
                                        sb0[:, hs], lhsT=kk[0:64, ksl],
                                        rhs=qq[0:64, qs], start=True,
                                        stop=(dj < 0), tile_position=(0, 0))
                                    nc.tensor.matmul(
                                        sb1[:, hs], lhsT=kk[64:128, ksl],
                                        rhs=qq[64:128, qs], start=True,
                                        stop=(dj < 0), tile_position=(64, 0))
                                    if dj >= 0:
                                        bm = bmask[:, dj * 512:(dj + 1) * 512]
                                        nc.tensor.matmul(
                                            sb0[:, hs], lhsT=ident[:], rhs=bm,
                                            start=False, stop=True)
                                        nc.tensor.matmul(
                                            sb1[:, hs], lhsT=ident[:], rhs=bm,
                                            start=False, stop=True)
                                p0 = pp.tile([128, 1024], bf16, tag="p",
                                             name=f"p0_{jj}_{qT}_{kp}")
                                p1 = pp.tile([128, 1024], bf16, tag="p",
                                             name=f"p1_{jj}_{qT}_{kp}")
                                nc.scalar.activation(
                                    p0[:], sb0[:],
                                    mybir.ActivationFunctionType.Exp)
                                nc.scalar.activation(
                                    p1[:], sb1[:],
                                    mybir.ActivationFunctionType.Exp)
                                for h in range(2):
                                    kt = 2 * kp + h
                                    hs = slice(h * 512, (h + 1) * 512)
                                    nc.tensor.matmul(
                                        o0[:],
                                        lhsT=v2[:, kt * 256:kt * 256 + 128],
                                        rhs=p0[:, hs], start=(kt == 0),
                                        stop=(kt == nkt - 1))
                                    nc.tensor.matmul(
                                        o1[:],
                                        lhsT=v2[:, kt * 256 + 128:
                                                (kt + 1) * 256],
                                        rhs=p1[:, hs], start=(kt == 0),
                                        stop=(kt == nkt - 1))
                            # normalize (approx-recip full tile; rows 64:128
                            # hold the replicated sums - base!=0 slices break
                            # the custom-DVE op)
                            rc = pp.tile([128, 512], f32, tag="rc", bufs=2,
                                         name=f"rc_{jj}_{qT}")
                            nc.vector.reciprocal_approx_fast(rc[:], o0[:])
                            nc.vector.tensor_tensor(
                                ao[0:64, qs], o0[0:64, :], rc[64:128, :],
                                mybir.AluOpType.mult)
                            rc2 = pp.tile([128, 512], f32, tag="rc", bufs=2,
                                          name=f"rc2_{jj}_{qT}")
                            nc.vector.reciprocal_approx_fast(rc2[:], o1[:])
                            nc.vector.tensor_tensor(
                                ao[64:128, qs], o1[0:64, :], rc2[64:128, :],
                                mybir.AluOpType.mult)

                def ag_half(hh):
                    for g in range(2):
                        for jj in (2 * hh, 2 * hh + 1):
                            nc.sync.dma_start(
                                bounces[hh][128 * g + 64 * (jj - 2 * hh):
                                            128 * g + 64 * (jj - 2 * hh) + 64,
                                            :],
                                ao[g * 64:(g + 1) * 64, jj * L:(jj + 1) * L])
                    nc.gpsimd.collective_compute(
                        "AllGather", mybir.AluOpType.bypass,
                        replica_groups=[[0, 1, 2, 3], [4, 5, 6, 7]],
                        ins=[bounces[hh].opt()], outs=[gath[hh].opt()])

                attention_half(0)
                ag_half(0)
                # o-proj first half: k-chunks 0..7 (features of AG1) -> stA
                aoks = {}
                for tt in range(TT):
                    ts = slice(tt * 512, (tt + 1) * 512)
                    for c in range(8):
                        aok = aogp.tile([128, 512], bf16, tag="aok",
                                        name=f"aok{tt}_{c}")
                        nc.sync.dma_start(
                            aok[:], gath[0][c * 128:(c + 1) * 128, ts])
                        aoks[(tt, c)] = aok
                stas = {}
                for tt in range(TT):
                    for ct in range(4):
                        psoA = ps.tile([128, 512], f32, tag="mm",
                                       name=f"psoA{tt}_{ct}")
                        for c in range(8):
                            nc.tensor.matmul(
                                psoA[:],
                                lhsT=wo_sb[:, c * 512 + ct * 128:
                                           c * 512 + (ct + 1) * 128],
                                rhs=aoks[(tt, c)][:], start=(c == 0),
                                stop=(c == 7))
                        sta = stap.tile([128, 512], f32, tag="sta",
                                        name=f"sta{tt}_{ct}")
                        nc.scalar.copy(sta[:], psoA[:])
                        stas[(tt, ct)] = sta

                attention_half(1)
                ag_half(1)
                # o-proj second half: k-chunks 8..15 + combine with stA
                for tt in range(TT):
                    ts = slice(tt * 512, (tt + 1) * 512)
                    aoks2 = []
                    for c in range(8):
                        aok2 = aogp.tile([128, 512], bf16, tag="aok",
                                         name=f"aok2_{tt}_{c}")
                        nc.sync.dma_start(
                            aok2[:], gath[1][c * 128:(c + 1) * 128, ts])
                        aoks2.append(aok2)
                    for ct in range(4):
                        psoB = ps.tile([128, 512], f32, tag="mm",
                                       name=f"psoB{tt}_{ct}")
                        for c in range(8):
                            nc.tensor.matmul(
                                psoB[:],
                                lhsT=wo_sb[:, (c + 8) * 512 + ct * 128:
                                           (c + 8) * 512 + (ct + 1) * 128],
                                rhs=aoks2[c][:], start=(c == 0),
                                stop=(c == 7))
                        ost = ostp.tile([128, 512], f32, tag="ost",
                                        name=f"ost{tt}_{ct}")
                        nc.vector.tensor_tensor(ost[:], psoB[:],
                                                stas[(tt, ct)][:],
                                                mybir.AluOpType.add)
                        nc.sync.dma_start(
                            out[ct * 128:(ct + 1) * 128, ts], ost[:])

            if dbg:
                src = {"qq": qq, "kk": kk, "v2": v2, "ao": ao}[dbg]
                nc.sync.dma_start(dbg_t[:], src[:])

    nc.compile()
    return nc


def _host_prep(hidden_states, cos, sin, Wq, Wk, Wv, Wo):
    """Build the 8 per-core input maps (all host-side slicing/transposes)."""
    scale = float(D) ** -0.5
    # rope coefficient tables [128, L]: 4 groups of 32 rows (d 0:32 pattern)
    cosT = cos[:, :32].T.astype(np.float32)          # [32, L]
    sinT = sin[:, :32].T.astype(np.float32)
    c1 = np.tile(cosT, (4, 1))                       # [128, L]
    c2 = np.concatenate([-sinT, sinT, -sinT, sinT], axis=0)
    tables = {
        "C1q": (c1 * scale).astype(BF16), "C2q": (c2 * scale).astype(BF16),
        "C1k": c1.astype(BF16), "C2k": c2.astype(BF16),
    }
    xTb = [np.ascontiguousarray(hidden_states[b].T).astype(BF16)
           for b in range(B)]
    in_maps = []
    for i in range(N_CORES):
        b, r = divmod(i, TP)
        # Wq rows reordered: M-tile m = heads (8r+m, 8r+4+m)
        rows = []
        for m in range(4):
            rows.append(Wq[(8 * r + m) * D:(8 * r + m + 1) * D])
            rows.append(Wq[(8 * r + 4 + m) * D:(8 * r + 4 + m + 1) * D])
        WqT_i = np.ascontiguousarray(np.concatenate(rows, 0).T).astype(BF16)
        WkT_i = np.ascontiguousarray(
            Wk[2 * r * D:(2 * r + 2) * D].T).astype(BF16)
        WvT_i = np.ascontiguousarray(
            Wv[2 * r * D:(2 * r + 2) * D].T).astype(BF16)
        # o-proj k-rows ordered to match the two gathered buffers:
        # half h row R: rank=R//256, g=(R%256)//128, jj=2h+(R%128)//64, d=R%64
        RR = np.arange(1024)
        perm = []
        for h in range(2):
            f = ((8 * (RR // 256) + 4 * ((RR % 256) // 128)
                  + 2 * h + (RR % 128) // 64) * D + RR % 64)
            perm.append(f)
        perm = np.concatenate(perm)
        WoT_i = np.ascontiguousarray(
            Wo[CW * r:CW * (r + 1), :].T[perm]).astype(BF16)
        in_maps.append({
            "xT": xTb[b], "WqT": WqT_i, "WkT": WkT_i, "WvT": WvT_i,
            "WoT": WoT_i, **tables,
        })
    return in_maps


def kernel(hidden_states, cos, sin, Wq, Wk, Wv, Wo, _want_profile=False):
    from concourse.bass_utils import run_bass_kernel_spmd

    if "nc" not in _cache:
        _cache["nc"] = _build_graph()
    nc = _cache["nc"]
    in_maps = _host_prep(np.asarray(hidden_states), np.asarray(cos),
                         np.asarray(sin), np.asarray(Wq), np.asarray(Wk),
                         np.asarray(Wv), np.asarray(Wo))
    res = run_bass_kernel_spmd(nc, in_maps, list(range(N_CORES)),
                               trace=_want_profile)
    # assemble: core (b, r) holds out^T [512, L] = cols [512r, 512r+512) of b
    full = np.empty((B, L, HID), np.float32)
    for i in range(N_CORES):
        b, r = divmod(i, TP)
        full[b, :, CW * r:CW * (r + 1)] = res.results[i]["out"].T
    if _want_profile:
        return full, res
    return full



# revision 5
# speedup vs baseline: 1.0891x; 1.0891x over previous
"""Distributed GQA attention kernel for one TRN2 chip (8 NeuronCores).

Problem: B=2, L=2048, HID=2048, H=32 q-heads, HKV=8 kv-heads, D=64,
rotary embedding, causal softmax, o-proj.

Sharding: core i -> batch b=i//4, TP rank r=i%4.  Each core computes
8 q-heads / 2 kv-heads of its batch, all-gathers the attention outputs
(feature-major, bf16) within its 4-core TP group, then computes its
512 output columns of the o-proj.  Host assembles the full output.

v2 restructure vs v1:
- Attention for head-pairs jj=0,1 is interleaved with the QKV
  projection loop (attn for q-tile tt right after proj tt), so the
  scalar-engine exp stream overlaps projection matmuls.
- The AllGather is split 4 ways (head-half x token-half) and each
  piece fires as soon as its attention chunk completes; o-proj chunks
  are emitted late so the PE never waits on a collective.
- PSUM->SBUF copies, o-proj partial-sum staging and normalization run
  on the vector engine; the scalar engine does exp only.
- Small warmup matmuls keep the PE HAM warm through the initial DMA.
- RoPE scale (D^-1/2) folded into Wq host-side (one cos/sin table pair).

All matmuls run in bf16 with fp32 PSUM accumulation.  Softmax skips the
row-max (logits are bounded ~|6| for these input scales) and obtains
row sums for free by appending a 64-wide ones block to V's stationary
operand; normalization is a DVE reciprocal + multiply.
"""

import sys

sys.path.insert(0, "/opt/trn_rl_repo")

import numpy as np
import ml_dtypes

B, L, HID = 2, 2048, 2048
H, HKV, D = 32, 8, 64
N_CORES = 8
TP = 4           # tensor-parallel group size
HL = 8           # q heads per core
CW = 512         # o-proj output columns per core
TT = 4           # t tiles of 512 over L
CCH = HID // 128 # contraction chunks (16)
BF16 = ml_dtypes.bfloat16

_cache = {}


def _build_graph():
    import concourse.bass as bass
    import concourse.tile as tile
    from concourse import bacc, mybir

    dt = mybir.dt
    f32, bf16 = dt.float32, dt.bfloat16

    nc = bacc.Bacc("TRN2", target_bir_lowering=False, debug=False,
                   num_devices=N_CORES)

    xT = nc.dram_tensor("xT", [HID, L], bf16, kind="ExternalInput")
    WqT = nc.dram_tensor("WqT", [HID, HL * D], bf16, kind="ExternalInput")
    WkT = nc.dram_tensor("WkT", [HID, 128], bf16, kind="ExternalInput")
    WvT = nc.dram_tensor("WvT", [HID, 128], bf16, kind="ExternalInput")
    WoT = nc.dram_tensor("WoT", [HID, CW], bf16, kind="ExternalInput")
    C1 = nc.dram_tensor("C1", [128, L], bf16, kind="ExternalInput")
    C2 = nc.dram_tensor("C2", [128, L], bf16, kind="ExternalInput")
    out = nc.dram_tensor("out", [CW, L], f32, kind="ExternalOutput")

    def bcast_m(ap2d, n):
        # [P, F] -> [P, n, F] with a step-0 middle dim (free-dim broadcast)
        return bass.AP(ap2d.tensor, ap2d.offset,
                       [ap2d.ap[0], [0, n], ap2d.ap[1]])

    with tile.TileContext(nc) as tc:
        with (
            tc.tile_pool(name="persist", bufs=1) as persist,
            tc.tile_pool(name="mm", bufs=2, space="PSUM") as pmm,
            tc.tile_pool(name="ps_s", bufs=2, space="PSUM") as ps_s,
            tc.tile_pool(name="po", bufs=2, space="PSUM") as po,
            tc.tile_pool(name="pp", bufs=4) as pp,
            tc.tile_pool(name="xt", bufs=2) as xtp,
            tc.tile_pool(name="rope", bufs=2) as rope,
            tc.tile_pool(name="aog", bufs=1) as aogp,
            tc.tile_pool(name="sta", bufs=16) as stap,
            tc.tile_pool(name="ost", bufs=2) as ostp,
            tc.tile_pool(name="dram", bufs=1, space="DRAM") as dram,
        ):
            # ---- persistent SBUF tensors ----
            qq = persist.tile([128, HL // 2 * L], bf16)      # roped Q^T, 2MB
            kk = persist.tile([128, L], bf16)                # roped K^T (2 kv)
            v2t = persist.tile([128, L], bf16)               # V^T staging
            v2 = persist.tile([128, CCH * 256], bf16)        # [V|1|V|1] per kt
            ao = persist.tile([128, HL // 2 * L], bf16)      # attn out^T
            c1 = persist.tile([128, L], bf16)
            c2 = persist.tile([128, L], bf16)
            wq_sb = persist.tile([128, CCH * 512], bf16)
            wk_sb = persist.tile([128, CCH * 128], bf16)
            wv_sb = persist.tile([128, CCH * 128], bf16)
            wo_sb = persist.tile([128, CCH * 512], bf16)
            warm = persist.tile([128, 256], bf16)

            # ---- warmup matmuls: keep the PE busy during initial DMA ----
            nc.gpsimd.memset(warm[:], 0.25)
            for i in range(20):
                psw = pmm.tile([128, 256], f32, tag="mm", name=f"warm{i}")
                nc.tensor.matmul(psw[:], lhsT=warm[:, 0:128], rhs=warm[:],
                                 start=True, stop=True)

            # ---- input DMAs (weights on sync; tables + wo on gpsimd) ----
            wq_v = WqT[:].rearrange("(c p) m -> p c m", p=128)
            wq_s = wq_sb[:].rearrange("p (c m) -> p c m", m=512)
            for cb in range(4):
                nc.sync.dma_start(wq_s[:, 4 * cb:4 * (cb + 1)],
                                  wq_v[:, 4 * cb:4 * (cb + 1)])
            nc.sync.dma_start(
                wk_sb[:].rearrange("p (c m) -> p c m", m=128),
                WkT[:].rearrange("(c p) m -> p c m", p=128))
            nc.sync.dma_start(
                wv_sb[:].rearrange("p (c m) -> p c m", m=128),
                WvT[:].rearrange("(c p) m -> p c m", p=128))
            nc.gpsimd.dma_start(c1[:], C1[:])
            nc.gpsimd.dma_start(c2[:], C2[:])
            nc.gpsimd.dma_start(
                wo_sb[:].rearrange("p (c m) -> p c m", m=512),
                WoT[:].rearrange("(c p) m -> p c m", p=128))

            # ones blocks of v2 (columns 64:128 and 192:256 of each kt group)
            for off in (64, 192):
                ones_view = bass.AP(v2.tensor, v2.offset + off,
                                    [v2.ap[0], [256, CCH], [1, 64]])
                nc.gpsimd.memset(ones_view, 1.0)

            # causal-mask helpers: ident for the mask matmul, bmask holds
            # -48 where q < 128*dj + k (dj = kt - 4*qT >= 0 diagonal block)
            ident = persist.tile([128, 128], bf16)
            nc.gpsimd.memset(ident[:], 1.0)
            nc.gpsimd.affine_select(
                out=ident[:], in_=ident[:], pattern=[[-1, 128]],
                compare_op=mybir.AluOpType.is_equal, fill=0.0,
                base=0, channel_multiplier=1)
            bmask = persist.tile([128, 4 * 512], bf16)
            nc.gpsimd.memset(bmask[:], -48.0)
            for j in range(4):
                nc.gpsimd.affine_select(
                    out=bmask[:, j * 512:(j + 1) * 512],
                    in_=bmask[:, j * 512:(j + 1) * 512],
                    pattern=[[-1, 512]], compare_op=mybir.AluOpType.is_gt,
                    fill=0.0, base=128 * j, channel_multiplier=1)

            xT_view = xT[:].rearrange("(c p) t -> p c t", p=128)

            def proj(tt):
                ts = slice(tt * 512, (tt + 1) * 512)
                xt = xtp.tile([128, CCH * 512], bf16, tag="xt")
                xt_s = xt[:].rearrange("p (c t) -> p c t", t=512)
                for cb in range(4):
                    nc.sync.dma_start(xt_s[:, 4 * cb:4 * (cb + 1)],
                                      xT_view[:, 4 * cb:4 * (cb + 1), ts])

                # --- Q: 4 M-tiles (head pair (jj, jj+4) each) ---
                qraw = rope.tile([128, 4 * 512], bf16, tag="qraw")
                for m in range(4):
                    psq = pmm.tile([128, 512], f32, tag="mm")
                    for c in range(CCH):
                        nc.tensor.matmul(
                            psq[:],
                            lhsT=wq_sb[:, c * 512 + m * 128:
                                       c * 512 + (m + 1) * 128],
                            rhs=xt[:, c * 512:(c + 1) * 512],
                            start=(c == 0), stop=(c == CCH - 1))
                    nc.vector.tensor_copy(qraw[:, m * 512:(m + 1) * 512],
                                          psq[:])

                # --- K ---
                kraw = rope.tile([128, 512], bf16, tag="kraw")
                psk = pmm.tile([128, 512], f32, tag="mm")
                for c in range(CCH):
                    nc.tensor.matmul(
                        psk[:], lhsT=wk_sb[:, c * 128:(c + 1) * 128],
                        rhs=xt[:, c * 512:(c + 1) * 512],
                        start=(c == 0), stop=(c == CCH - 1))
                nc.vector.tensor_copy(kraw[:], psk[:])

                # --- V ---
                psv = pmm.tile([128, 512], f32, tag="mm")
                for c in range(CCH):
                    nc.tensor.matmul(
                        psv[:], lhsT=wv_sb[:, c * 128:(c + 1) * 128],
                        rhs=xt[:, c * 512:(c + 1) * 512],
                        start=(c == 0), stop=(c == CCH - 1))
                nc.vector.tensor_copy(v2t[:, ts], psv[:])

                # --- RoPE on Q (in-place, no qtmp) ---
                qsw = rope.tile([128, 4 * 512], bf16, tag="qsw")
                for a, b_ in ((0, 32), (32, 0), (64, 96), (96, 64)):
                    nc.gpsimd.dma_start(qsw[b_:b_ + 32, :], qraw[a:a + 32, :])
                q3 = qraw[:].rearrange("p (m t) -> p m t", t=512)
                s3 = qsw[:].rearrange("p (m t) -> p m t", t=512)
                qqd = bass.AP(qq.tensor, qq.offset + tt * 512,
                              [qq.ap[0], [2048, 4], [1, 512]])
                nc.vector.tensor_tensor(qqd, q3, bcast_m(c1[:, ts], 4),
                                        mybir.AluOpType.mult)
                nc.vector.tensor_tensor(s3, s3, bcast_m(c2[:, ts], 4),
                                        mybir.AluOpType.mult)
                nc.vector.tensor_tensor(qqd, qqd, s3, mybir.AluOpType.add)

                # --- RoPE on K (in-place, no ktmp) ---
                ksw = rope.tile([128, 512], bf16, tag="ksw")
                for a, b_ in ((0, 32), (32, 0), (64, 96), (96, 64)):
                    nc.gpsimd.dma_start(ksw[b_:b_ + 32, :], kraw[a:a + 32, :])
                nc.vector.tensor_tensor(kraw[:], kraw[:], c1[:, ts],
                                        mybir.AluOpType.mult)
                nc.vector.tensor_tensor(ksw[:], ksw[:], c2[:, ts],
                                        mybir.AluOpType.mult)
                nc.vector.tensor_tensor(kk[:, ts], kraw[:], ksw[:],
                                        mybir.AluOpType.add)

                # --- V transpose to token-major (v2 cols 0:64/128:192) ---
                for g in range(2):
                    v2_dst = bass.AP(v2.tensor,
                                     v2.offset + (4 * tt) * 256 + g * 128,
                                     [v2.ap[0], [256, 4], [1, 64]])
                    nc.sync.dma_start_transpose(
                        v2_dst, v2t[g * 64:(g + 1) * 64, ts])

            def attn_group(jj, qT):
                """Attention for head pair (jj, jj+4), query tile qT."""
                nkt = 4 * qT + 4
                qoff = jj * L
                qs = slice(qoff + qT * 512, qoff + (qT + 1) * 512)
                o0 = po.tile([128, 512], f32, tag="o", name=f"o0_{jj}_{qT}")
                o1 = po.tile([128, 512], f32, tag="o", name=f"o1_{jj}_{qT}")
                for kp in range(nkt // 2):
                    sb0 = ps_s.tile([128, 1024], f32, tag="s",
                                    name=f"sb0_{jj}_{qT}_{kp}")
                    sb1 = ps_s.tile([128, 1024], f32, tag="s",
                                    name=f"sb1_{jj}_{qT}_{kp}")
                    for h in range(2):
                        kt = 2 * kp + h
                        ksl = slice(kt * 128, (kt + 1) * 128)
                        hs = slice(h * 512, (h + 1) * 512)
                        dj = kt - 4 * qT  # >=0 -> diagonal block
                        nc.tensor.matmul(
                            sb0[:, hs], lhsT=kk[0:64, ksl],
                            rhs=qq[0:64, qs], start=True,
                            stop=(dj < 0), tile_position=(0, 0))
                        nc.tensor.matmul(
                            sb1[:, hs], lhsT=kk[64:128, ksl],
                            rhs=qq[64:128, qs], start=True,
                            stop=(dj < 0), tile_position=(64, 0))
                        if dj >= 0:
                            bm = bmask[:, dj * 512:(dj + 1) * 512]
                            nc.tensor.matmul(
                                sb0[:, hs], lhsT=ident[:], rhs=bm,
                                start=False, stop=True)
                            nc.tensor.matmul(
                                sb1[:, hs], lhsT=ident[:], rhs=bm,
                                start=False, stop=True)
                    p0 = pp.tile([128, 1024], bf16, tag="p",
                                 name=f"p0_{jj}_{qT}_{kp}")
                    p1 = pp.tile([128, 1024], bf16, tag="p",
                                 name=f"p1_{jj}_{qT}_{kp}")
                    nc.scalar.activation(
                        p0[:], sb0[:], mybir.ActivationFunctionType.Exp)
                    nc.scalar.activation(
                        p1[:], sb1[:], mybir.ActivationFunctionType.Exp)
                    for h in range(2):
                        kt = 2 * kp + h
                        hs = slice(h * 512, (h + 1) * 512)
                        nc.tensor.matmul(
                            o0[:], lhsT=v2[:, kt * 256:kt * 256 + 128],
                            rhs=p0[:, hs], start=(kt == 0),
                            stop=(kt == nkt - 1))
                        nc.tensor.matmul(
                            o1[:], lhsT=v2[:, kt * 256 + 128:(kt + 1) * 256],
                            rhs=p1[:, hs], start=(kt == 0),
                            stop=(kt == nkt - 1))
                # normalize (approx-recip full tile; rows 64:128 hold the
                # replicated sums - base!=0 slices break the custom-DVE op)
                rc = pp.tile([128, 512], f32, tag="rc", bufs=2,
                             name=f"rc_{jj}_{qT}")
                nc.vector.reciprocal_approx_fast(rc[:], o0[:])
                nc.vector.tensor_tensor(
                    ao[0:64, qs], o0[0:64, :], rc[64:128, :],
                    mybir.AluOpType.mult)
                rc2 = pp.tile([128, 512], f32, tag="rc", bufs=2,
                              name=f"rc2_{jj}_{qT}")
                nc.vector.reciprocal_approx_fast(rc2[:], o1[:])
                nc.vector.tensor_tensor(
                    ao[64:128, qs], o1[0:64, :], rc2[64:128, :],
                    mybir.AluOpType.mult)

            # 4 AG pieces: (head-half hh, token-half th)
            bounces = {}
            gaths = {}
            for hh in range(2):
                for th in range(2):
                    bounces[(hh, th)] = dram.tile([2 * 128, 1024], bf16,
                                                  name=f"bounce{hh}{th}")
                    gaths[(hh, th)] = dram.tile([TP * 2 * 128, 1024], bf16,
                                                name=f"gath{hh}{th}")

            def ship(hh, th):
                """Bounce ao (head pair hh, token half th) + AllGather."""
                bnc = bounces[(hh, th)]
                tsl = slice(th * 1024, (th + 1) * 1024)
                for g in range(2):
                    for jj in (2 * hh, 2 * hh + 1):
                        r0 = 128 * g + 64 * (jj - 2 * hh)
                        nc.sync.dma_start(
                            bnc[r0:r0 + 64, :],
                            ao[g * 64:(g + 1) * 64,
                               jj * L + th * 1024:jj * L + (th + 1) * 1024])
                nc.gpsimd.collective_compute(
                    "AllGather", mybir.AluOpType.bypass,
                    replica_groups=[[0, 1, 2, 3], [4, 5, 6, 7]],
                    ins=[bnc.opt()], outs=[gaths[(hh, th)].opt()])

            stas = {}

            def oproj(hh, th):
                """o-proj partial for gather chunk (hh, th): 2 token tiles."""
                aok = aogp.tile([128, 8 * 1024], bf16, tag="aok")
                nc.scalar.dma_start(
                    aok[:].rearrange("p (c t) -> p c t", t=1024),
                    gaths[(hh, th)][:].rearrange("(c p) t -> p c t", p=128))
                for tl in range(2):
                    tt = 2 * th + tl
                    ts = slice(tt * 512, (tt + 1) * 512)
                    for ct in range(4):
                        pso = pmm.tile([128, 512], f32, tag="mm",
                                       name=f"pso{hh}_{tt}_{ct}")
                        for c in range(8):
                            nc.tensor.matmul(
                                pso[:],
                                lhsT=wo_sb[:, (8 * hh + c) * 512 + ct * 128:
                                           (8 * hh + c) * 512 +
                                           (ct + 1) * 128],
                                rhs=aok[:, c * 1024 + tl * 512:
                                        c * 1024 + (tl + 1) * 512],
                                start=(c == 0), stop=(c == 7))
                        if hh == 0:
                            sta = stap.tile([128, 512], bf16, tag="sta",
                                            name=f"sta{tt}_{ct}")
                            nc.vector.tensor_copy(sta[:], pso[:])
                            stas[(tt, ct)] = sta
                        else:
                            ost = ostp.tile([128, 512], f32, tag="ost",
                                            name=f"ost{tt}_{ct}")
                            nc.vector.tensor_tensor(
                                ost[:], pso[:], stas[(tt, ct)][:],
                                mybir.AluOpType.add)
                            nc.scalar.dma_start(
                                out[ct * 128:(ct + 1) * 128, ts], ost[:])

            # ================= schedule =================
            for tt in range(TT):
                proj(tt)
                attn_group(0, tt)
                attn_group(1, tt)
                if tt == 1:
                    ship(0, 0)
            ship(0, 1)

            attn_group(2, 2)
            attn_group(3, 2)
            attn_group(2, 3)
            attn_group(3, 3)
            ship(1, 1)

            oproj(0, 0)

            attn_group(2, 0)
            attn_group(3, 0)
            attn_group(2, 1)
            attn_group(3, 1)
            ship(1, 0)

            oproj(0, 1)
            oproj(1, 1)
            oproj(1, 0)

    nc.compile()
    return nc


def _host_prep(hidden_states, cos, sin, Wq, Wk, Wv, Wo):
    """Build the 8 per-core input maps (all host-side slicing/transposes)."""
    scale = float(D) ** -0.5
    # rope coefficient tables [128, L]: 4 groups of 32 rows (d 0:32 pattern)
    cosT = cos[:, :32].T.astype(np.float32)          # [32, L]
    sinT = sin[:, :32].T.astype(np.float32)
    c1 = np.tile(cosT, (4, 1))                       # [128, L]
    c2 = np.concatenate([-sinT, sinT, -sinT, sinT], axis=0)
    tables = {"C1": c1.astype(BF16), "C2": c2.astype(BF16)}
    xTb = [np.ascontiguousarray(hidden_states[b].T).astype(BF16)
           for b in range(B)]
    in_maps = []
    for i in range(N_CORES):
        b, r = divmod(i, TP)
        # Wq rows reordered: M-tile m = heads (8r+m, 8r+4+m); scale folded in
        rows = []
        for m in range(4):
            rows.append(Wq[(8 * r + m) * D:(8 * r + m + 1) * D])
            rows.append(Wq[(8 * r + 4 + m) * D:(8 * r + 4 + m + 1) * D])
        WqT_i = np.ascontiguousarray(
            (np.concatenate(rows, 0) * scale).T).astype(BF16)
        WkT_i = np.ascontiguousarray(
            Wk[2 * r * D:(2 * r + 2) * D].T).astype(BF16)
        WvT_i = np.ascontiguousarray(
            Wv[2 * r * D:(2 * r + 2) * D].T).astype(BF16)
        # o-proj k-rows ordered to match the gathered buffers:
        # half h row R: rank=R//256, g=(R%256)//128, jj=2h+(R%128)//64, d=R%64
        RR = np.arange(1024)
        perm = []
        for h in range(2):
            f = ((8 * (RR // 256) + 4 * ((RR % 256) // 128)
                  + 2 * h + (RR % 128) // 64) * D + RR % 64)
            perm.append(f)
        perm = np.concatenate(perm)
        WoT_i = np.ascontiguousarray(
            Wo[CW * r:CW * (r + 1), :].T[perm]).astype(BF16)
        in_maps.append({
            "xT": xTb[b], "WqT": WqT_i, "WkT": WkT_i, "WvT": WvT_i,
            "WoT": WoT_i, **tables,
        })
    return in_maps


def kernel(hidden_states, cos, sin, Wq, Wk, Wv, Wo, _want_profile=False):
    from concourse.bass_utils import run_bass_kernel_spmd

    if "nc" not in _cache:
        _cache["nc"] = _build_graph()
    nc = _cache["nc"]
    in_maps = _host_prep(np.asarray(hidden_states), np.asarray(cos),
                         np.asarray(sin), np.asarray(Wq), np.asarray(Wk),
                         np.asarray(Wv), np.asarray(Wo))
    res = run_bass_kernel_spmd(nc, in_maps, list(range(N_CORES)),
                               trace=_want_profile)
    # assemble: core (b, r) holds out^T [512, L] = cols [512r, 512r+512) of b
    full = np.empty((B, L, HID), np.float32)
    for i in range(N_CORES):
        b, r = divmod(i, TP)
        full[b, :, CW * r:CW * (r + 1)] = res.results[i]["out"].T
    if _want_profile:
        return full, res
    return full


# revision 6
# speedup vs baseline: 1.2393x; 1.1379x over previous
"""Distributed GQA attention kernel for one TRN2 chip (8 NeuronCores).

Problem: B=2, L=2048, HID=2048, H=32 q-heads, HKV=8 kv-heads, D=64,
rotary embedding, causal softmax, o-proj.

Sharding: core i -> batch b=i//4, TP rank r=i%4.  Each core computes
8 q-heads / 2 kv-heads of its batch, all-gathers the attention outputs
(feature-major, bf16) within its 4-core TP group, then computes its
512 output columns of the o-proj.  Host assembles the full output.

Schedule (v3):
- proj(tt) and attention for head-pairs jj=0,1 interleave per q-tile,
  so the scalar-engine exp stream overlaps projection matmuls.
- AllGathers are split (h0: 2 token-halves, h1: token-half + 2 token-
  quarters) and fire as soon as their attention chunk completes; all
  collectives are emitted after the last DMA-transpose (xbar modes
  serialize against in-flight collectives).
- o-proj chunks are emitted where the PE has slack; the tail is one
  quarter-AG + 32 matmuls.
- All weight/x DRAM operands are host-packed so every load is one DMA
  with 128 contiguous per-partition descriptors.
- PSUM->SBUF copies, staging and normalization run on the vector
  engine; the scalar engine does exp (the attention pacer) only.

All matmuls run in bf16 with fp32 PSUM accumulation.  Softmax skips the
row-max (logits are bounded ~|6| for these input scales) and obtains
row sums for free by appending a 64-wide ones block to V's stationary
operand; normalization is a DVE reciprocal + multiply.
"""

import sys

sys.path.insert(0, "/opt/trn_rl_repo")

import numpy as np
import ml_dtypes

B, L, HID = 2, 2048, 2048
H, HKV, D = 32, 8, 64
N_CORES = 8
TP = 4           # tensor-parallel group size
HL = 8           # q heads per core
CW = 512         # o-proj output columns per core
TT = 4           # t tiles of 512 over L
CCH = HID // 128 # contraction chunks (16)
BF16 = ml_dtypes.bfloat16

_cache = {}


def _build_graph():
    import concourse.bass as bass
    import concourse.tile as tile
    from concourse import bacc, mybir

    dt = mybir.dt
    f32, bf16 = dt.float32, dt.bfloat16

    nc = bacc.Bacc("TRN2", target_bir_lowering=False, debug=False,
                   num_devices=N_CORES)

    # all packed [128, n] with per-partition-contiguous layout
    xP = nc.dram_tensor("xP", [128, TT * CCH * 512], bf16,
                        kind="ExternalInput")
    WqP = nc.dram_tensor("WqP", [128, CCH * 512], bf16, kind="ExternalInput")
    WkP = nc.dram_tensor("WkP", [128, CCH * 128], bf16, kind="ExternalInput")
    WvP = nc.dram_tensor("WvP", [128, CCH * 128], bf16, kind="ExternalInput")
    WoP = nc.dram_tensor("WoP", [128, CCH * 512], bf16, kind="ExternalInput")
    C1 = nc.dram_tensor("C1", [128, L], bf16, kind="ExternalInput")
    C2 = nc.dram_tensor("C2", [128, L], bf16, kind="ExternalInput")
    out = nc.dram_tensor("out", [CW, L], f32, kind="ExternalOutput")

    def bcast_m(ap2d, n):
        # [P, F] -> [P, n, F] with a step-0 middle dim (free-dim broadcast)
        return bass.AP(ap2d.tensor, ap2d.offset,
                       [ap2d.ap[0], [0, n], ap2d.ap[1]])

    with tile.TileContext(nc) as tc:
        with (
            tc.tile_pool(name="persist", bufs=1) as persist,
            tc.tile_pool(name="mm", bufs=2, space="PSUM") as pmm,
            tc.tile_pool(name="ps_s", bufs=2, space="PSUM") as ps_s,
            tc.tile_pool(name="po", bufs=2, space="PSUM") as po,
            tc.tile_pool(name="pp", bufs=4) as pp,
            tc.tile_pool(name="xt", bufs=2) as xtp,
            tc.tile_pool(name="rope", bufs=2) as rope,
            tc.tile_pool(name="aog", bufs=1) as aogp,
            tc.tile_pool(name="sta", bufs=16) as stap,
            tc.tile_pool(name="ost", bufs=2) as ostp,
            tc.tile_pool(name="dram", bufs=1, space="DRAM") as dram,
        ):
            # ---- persistent SBUF tensors ----
            qq = persist.tile([128, HL // 2 * L], bf16)      # roped Q^T, 2MB
            kk = persist.tile([128, L], bf16)                # roped K^T (2 kv)
            v2t = persist.tile([128, L], bf16)               # V^T staging
            v2 = persist.tile([128, CCH * 256], bf16)        # [V|1|V|1] per kt
            ao = persist.tile([128, HL // 2 * L], bf16)      # attn out^T
            c1 = persist.tile([128, L], bf16)
            c2 = persist.tile([128, L], bf16)
            wq_sb = persist.tile([128, CCH * 512], bf16)
            wk_sb = persist.tile([128, CCH * 128], bf16)
            wv_sb = persist.tile([128, CCH * 128], bf16)
            wo_sb = persist.tile([128, CCH * 512], bf16)
            warm = persist.tile([128, 256], bf16)

            # ---- warmup matmuls: keep the PE busy during initial DMA ----
            nc.gpsimd.memset(warm[:], 0.25)
            for i in range(28):
                psw = pmm.tile([128, 256], f32, tag="mm", name=f"warm{i}")
                nc.tensor.matmul(psw[:], lhsT=warm[:, 0:128], rhs=warm[:],
                                 start=True, stop=True)

            # ---- input DMAs (single contiguous load each) ----
            nc.sync.dma_start(wq_sb[:], WqP[:])
            xP_v = xP[:].rearrange("p (tt f) -> p tt f", f=CCH * 512)
            xts = []
            xt0 = xtp.tile([128, CCH * 512], bf16, tag="xt", name="xt0")
            nc.sync.dma_start(xt0[:], xP_v[:, 0])
            xts.append(xt0)
            nc.sync.dma_start(wk_sb[:], WkP[:])
            nc.sync.dma_start(wv_sb[:], WvP[:])
            nc.gpsimd.dma_start(c1[:], C1[:])
            nc.gpsimd.dma_start(c2[:], C2[:])
            nc.gpsimd.dma_start(wo_sb[:], WoP[:])

            # ones blocks of v2 (columns 64:128 and 192:256 of each kt group)
            for off in (64, 192):
                ones_view = bass.AP(v2.tensor, v2.offset + off,
                                    [v2.ap[0], [256, CCH], [1, 64]])
                nc.gpsimd.memset(ones_view, 1.0)

            # causal-mask helpers: ident for the mask matmul, bmask holds
            # -48 where q < 128*dj + k (dj = kt - 4*qT >= 0 diagonal block)
            ident = persist.tile([128, 128], bf16)
            nc.gpsimd.memset(ident[:], 1.0)
            nc.gpsimd.affine_select(
                out=ident[:], in_=ident[:], pattern=[[-1, 128]],
                compare_op=mybir.AluOpType.is_equal, fill=0.0,
                base=0, channel_multiplier=1)
            bmask = persist.tile([128, 4 * 512], bf16)
            nc.gpsimd.memset(bmask[:], -48.0)
            for j in range(4):
                nc.gpsimd.affine_select(
                    out=bmask[:, j * 512:(j + 1) * 512],
                    in_=bmask[:, j * 512:(j + 1) * 512],
                    pattern=[[-1, 512]], compare_op=mybir.AluOpType.is_gt,
                    fill=0.0, base=128 * j, channel_multiplier=1)

            def proj(tt):
                ts = slice(tt * 512, (tt + 1) * 512)
                xt = xts[tt]
                if tt + 1 < TT:  # prefetch next tile
                    xtn = xtp.tile([128, CCH * 512], bf16, tag="xt",
                                   name=f"xt{tt + 1}")
                    nc.sync.dma_start(xtn[:], xP_v[:, tt + 1])
                    xts.append(xtn)

                # --- Q: 4 M-tiles (head pair (jj, jj+4) each) ---
                qraw = rope.tile([128, 4 * 512], bf16, tag="qraw")
                for m in range(4):
                    psq = pmm.tile([128, 512], f32, tag="mm")
                    for c in range(CCH):
                        nc.tensor.matmul(
                            psq[:],
                            lhsT=wq_sb[:, c * 512 + m * 128:
                                       c * 512 + (m + 1) * 128],
                            rhs=xt[:, c * 512:(c + 1) * 512],
                            start=(c == 0), stop=(c == CCH - 1))
                    nc.vector.tensor_copy(qraw[:, m * 512:(m + 1) * 512],
                                          psq[:])

                # --- K ---
                kraw = rope.tile([128, 512], bf16, tag="kraw")
                psk = pmm.tile([128, 512], f32, tag="mm")
                for c in range(CCH):
                    nc.tensor.matmul(
                        psk[:], lhsT=wk_sb[:, c * 128:(c + 1) * 128],
                        rhs=xt[:, c * 512:(c + 1) * 512],
                        start=(c == 0), stop=(c == CCH - 1))
                nc.vector.tensor_copy(kraw[:], psk[:])

                # --- V ---
                psv = pmm.tile([128, 512], f32, tag="mm")
                for c in range(CCH):
                    nc.tensor.matmul(
                        psv[:], lhsT=wv_sb[:, c * 128:(c + 1) * 128],
                        rhs=xt[:, c * 512:(c + 1) * 512],
                        start=(c == 0), stop=(c == CCH - 1))
                nc.vector.tensor_copy(v2t[:, ts], psv[:])

                # --- RoPE on Q (in-place, no qtmp) ---
                qsw = rope.tile([128, 4 * 512], bf16, tag="qsw")
                for a, b_ in ((0, 32), (32, 0), (64, 96), (96, 64)):
                    nc.gpsimd.dma_start(qsw[b_:b_ + 32, :], qraw[a:a + 32, :])
                q3 = qraw[:].rearrange("p (m t) -> p m t", t=512)
                s3 = qsw[:].rearrange("p (m t) -> p m t", t=512)
                qqd = bass.AP(qq.tensor, qq.offset + tt * 512,
                              [qq.ap[0], [2048, 4], [1, 512]])
                nc.vector.tensor_tensor(qqd, q3, bcast_m(c1[:, ts], 4),
                                        mybir.AluOpType.mult)
                nc.vector.tensor_tensor(s3, s3, bcast_m(c2[:, ts], 4),
                                        mybir.AluOpType.mult)
                nc.vector.tensor_tensor(qqd, qqd, s3, mybir.AluOpType.add)

                # --- RoPE on K (in-place, no ktmp) ---
                ksw = rope.tile([128, 512], bf16, tag="ksw")
                for a, b_ in ((0, 32), (32, 0), (64, 96), (96, 64)):
                    nc.gpsimd.dma_start(ksw[b_:b_ + 32, :], kraw[a:a + 32, :])
                nc.vector.tensor_tensor(kraw[:], kraw[:], c1[:, ts],
                                        mybir.AluOpType.mult)
                nc.vector.tensor_tensor(ksw[:], ksw[:], c2[:, ts],
                                        mybir.AluOpType.mult)
                nc.vector.tensor_tensor(kk[:, ts], kraw[:], ksw[:],
                                        mybir.AluOpType.add)

                # --- V transpose to token-major (v2 cols 0:64/128:192) ---
                for g in range(2):
                    v2_dst = bass.AP(v2.tensor,
                                     v2.offset + (4 * tt) * 256 + g * 128,
                                     [v2.ap[0], [256, 4], [1, 64]])
                    nc.sync.dma_start_transpose(
                        v2_dst, v2t[g * 64:(g + 1) * 64, ts])

            def attn_group(jj, qT):
                """Attention for head pair (jj, jj+4), query tile qT."""
                nkt = 4 * qT + 4
                qoff = jj * L
                qs = slice(qoff + qT * 512, qoff + (qT + 1) * 512)
                o0 = po.tile([128, 512], f32, tag="o", name=f"o0_{jj}_{qT}")
                o1 = po.tile([128, 512], f32, tag="o", name=f"o1_{jj}_{qT}")
                for kp in range(nkt // 2):
                    sb0 = ps_s.tile([128, 1024], f32, tag="s",
                                    name=f"sb0_{jj}_{qT}_{kp}")
                    sb1 = ps_s.tile([128, 1024], f32, tag="s",
                                    name=f"sb1_{jj}_{qT}_{kp}")
                    for h in range(2):
                        kt = 2 * kp + h
                        ksl = slice(kt * 128, (kt + 1) * 128)
                        hs = slice(h * 512, (h + 1) * 512)
                        dj = kt - 4 * qT  # >=0 -> diagonal block
                        nc.tensor.matmul(
                            sb0[:, hs], lhsT=kk[0:64, ksl],
                            rhs=qq[0:64, qs], start=True,
                            stop=(dj < 0), tile_position=(0, 0))
                        nc.tensor.matmul(
                            sb1[:, hs], lhsT=kk[64:128, ksl],
                            rhs=qq[64:128, qs], start=True,
                            stop=(dj < 0), tile_position=(64, 0))
                        if dj >= 0:
                            bm = bmask[:, dj * 512:(dj + 1) * 512]
                            nc.tensor.matmul(
                                sb0[:, hs], lhsT=ident[:], rhs=bm,
                                start=False, stop=True)
                            nc.tensor.matmul(
                                sb1[:, hs], lhsT=ident[:], rhs=bm,
                                start=False, stop=True)
                    p0 = pp.tile([128, 1024], bf16, tag="p",
                                 name=f"p0_{jj}_{qT}_{kp}")
                    p1 = pp.tile([128, 1024], bf16, tag="p",
                                 name=f"p1_{jj}_{qT}_{kp}")
                    nc.scalar.activation(
                        p0[:], sb0[:], mybir.ActivationFunctionType.Exp)
                    nc.scalar.activation(
                        p1[:], sb1[:], mybir.ActivationFunctionType.Exp)
                    for h in range(2):
                        kt = 2 * kp + h
                        hs = slice(h * 512, (h + 1) * 512)
                        nc.tensor.matmul(
                            o0[:], lhsT=v2[:, kt * 256:kt * 256 + 128],
                            rhs=p0[:, hs], start=(kt == 0),
                            stop=(kt == nkt - 1))
                        nc.tensor.matmul(
                            o1[:], lhsT=v2[:, kt * 256 + 128:(kt + 1) * 256],
                            rhs=p1[:, hs], start=(kt == 0),
                            stop=(kt == nkt - 1))
                # normalize (approx-recip full tile; rows 64:128 hold the
                # replicated sums - base!=0 slices break the custom-DVE op)
                rc = pp.tile([128, 512], f32, tag="rc", bufs=2,
                             name=f"rc_{jj}_{qT}")
                nc.vector.reciprocal_approx_fast(rc[:], o0[:])
                nc.vector.tensor_tensor(
                    ao[0:64, qs], o0[0:64, :], rc[64:128, :],
                    mybir.AluOpType.mult)
                rc2 = pp.tile([128, 512], f32, tag="rc", bufs=2,
                              name=f"rc2_{jj}_{qT}")
                nc.vector.reciprocal_approx_fast(rc2[:], o1[:])
                nc.vector.tensor_tensor(
                    ao[64:128, qs], o1[0:64, :], rc2[64:128, :],
                    mybir.AluOpType.mult)

            # AG pieces: (head-half hh, token range [t0, t1) in 512-tiles)
            bounces = {}
            gaths = {}
            pieces = [(0, 0, 2), (0, 2, 4), (1, 0, 2), (1, 2, 3), (1, 3, 4)]
            for hh, t0, t1 in pieces:
                w = (t1 - t0) * 512
                bounces[(hh, t0)] = dram.tile([2 * 128, w], bf16,
                                              name=f"bounce{hh}_{t0}")
                gaths[(hh, t0)] = dram.tile([TP * 2 * 128, w], bf16,
                                            name=f"gath{hh}_{t0}")

            def ship(hh, t0, t1):
                """Bounce ao (head pair hh, token tiles [t0,t1)) + AG."""
                bnc = bounces[(hh, t0)]
                w = (t1 - t0) * 512
                for g in range(2):
                    for jj in (2 * hh, 2 * hh + 1):
                        r0 = 128 * g + 64 * (jj - 2 * hh)
                        nc.sync.dma_start(
                            bnc[r0:r0 + 64, :],
                            ao[g * 64:(g + 1) * 64,
                               jj * L + t0 * 512:jj * L + t1 * 512])
                nc.gpsimd.collective_compute(
                    "AllGather", mybir.AluOpType.bypass,
                    replica_groups=[[0, 1, 2, 3], [4, 5, 6, 7]],
                    ins=[bnc.opt()], outs=[gaths[(hh, t0)].opt()])

            stas = {}

            def oproj(hh, t0, t1):
                """o-proj partial for gather piece (hh, [t0,t1))."""
                w = (t1 - t0) * 512
                aok = aogp.tile([128, 8 * 1024], bf16, tag="aok",
                                name=f"aok{hh}_{t0}")
                av = bass.AP(aok.tensor, aok.offset,
                             [aok.ap[0], [w, 8], [1, w]])
                nc.scalar.dma_start(
                    av, gaths[(hh, t0)][:].rearrange("(c p) t -> p c t",
                                                     p=128))
                for tt in range(t0, t1):
                    tl = tt - t0
                    ts = slice(tt * 512, (tt + 1) * 512)
                    for ct in range(4):
                        pso = pmm.tile([128, 512], f32, tag="mm",
                                       name=f"pso{hh}_{tt}_{ct}")
                        for c in range(8):
                            nc.tensor.matmul(
                                pso[:],
                                lhsT=wo_sb[:, (8 * hh + c) * 512 + ct * 128:
                                           (8 * hh + c) * 512 +
                                           (ct + 1) * 128],
                                rhs=aok[:, c * w + tl * 512:
                                        c * w + (tl + 1) * 512],
                                start=(c == 0), stop=(c == 7))
                        if hh == 0:
                            sta = stap.tile([128, 512], bf16, tag="sta",
                                            name=f"sta{tt}_{ct}")
                            nc.vector.tensor_copy(sta[:], pso[:])
                            stas[(tt, ct)] = sta
                        else:
                            ost = ostp.tile([128, 512], f32, tag="ost",
                                            name=f"ost{tt}_{ct}")
                            nc.vector.tensor_tensor(
                                ost[:], pso[:], stas[(tt, ct)][:],
                                mybir.AluOpType.add)
                            nc.scalar.dma_start(
                                out[ct * 128:(ct + 1) * 128, ts], ost[:])

            # ================= schedule =================
            for tt in range(TT):
                proj(tt)
                attn_group(0, tt)
                attn_group(1, tt)
            # all DMA-transposes precede the first collective (xbar modes
            # serialize against in-flight collective SDMA traffic)
            ship(0, 0, 2)
            ship(0, 2, 4)

            attn_group(2, 0)
            attn_group(3, 0)
            attn_group(2, 1)
            attn_group(3, 1)
            ship(1, 0, 2)

            oproj(0, 0, 2)
            attn_group(2, 2)
            attn_group(3, 2)
            ship(1, 2, 3)

            oproj(0, 2, 4)
            attn_group(2, 3)
            attn_group(3, 3)
            ship(1, 3, 4)

            oproj(1, 0, 2)
            oproj(1, 2, 3)
            oproj(1, 3, 4)

    nc.compile()
    return nc


def _host_prep(hidden_states, cos, sin, Wq, Wk, Wv, Wo):
    """Build the 8 per-core input maps (all host-side packing)."""
    scale = float(D) ** -0.5
    # rope coefficient tables [128, L]: 4 groups of 32 rows (d 0:32 pattern)
    cosT = cos[:, :32].T.astype(np.float32)          # [32, L]
    sinT = sin[:, :32].T.astype(np.float32)
    c1 = np.tile(cosT, (4, 1))                       # [128, L]
    c2 = np.concatenate([-sinT, sinT, -sinT, sinT], axis=0)
    tables = {"C1": c1.astype(BF16), "C2": c2.astype(BF16)}

    def pack(WT, m):
        # WT [HID, m] -> [128, CCH*m] with row p = concat_c WT[c*128+p, :]
        return np.ascontiguousarray(
            WT.reshape(CCH, 128, m).transpose(1, 0, 2).reshape(128, CCH * m)
        ).astype(BF16)

    # x packed per (tt, c): [128, tt, c, 512]
    xPb = []
    for b in range(B):
        xT = hidden_states[b].T.astype(np.float32)   # [HID, L]
        xp = (xT.reshape(CCH, 128, TT, 512).transpose(1, 2, 0, 3)
              .reshape(128, TT * CCH * 512))
        xPb.append(np.ascontiguousarray(xp).astype(BF16))

    in_maps = []
    for i in range(N_CORES):
        b, r = divmod(i, TP)
        # Wq rows reordered: M-tile m = heads (8r+m, 8r+4+m); scale folded in
        rows = []
        for m in range(4):
            rows.append(Wq[(8 * r + m) * D:(8 * r + m + 1) * D])
            rows.append(Wq[(8 * r + 4 + m) * D:(8 * r + 4 + m + 1) * D])
        WqT_i = (np.concatenate(rows, 0) * scale).T.astype(np.float32)
        WkT_i = Wk[2 * r * D:(2 * r + 2) * D].T.astype(np.float32)
        WvT_i = Wv[2 * r * D:(2 * r + 2) * D].T.astype(np.float32)
        # o-proj k-rows ordered to match the gathered buffers:
        # half h row R: rank=R//256, g=(R%256)//128, jj=2h+(R%128)//64, d=R%64
        RR = np.arange(1024)
        perm = []
        for h in range(2):
            f = ((8 * (RR // 256) + 4 * ((RR % 256) // 128)
                  + 2 * h + (RR % 128) // 64) * D + RR % 64)
            perm.append(f)
        perm = np.concatenate(perm)
        WoT_i = Wo[CW * r:CW * (r + 1), :].T[perm].astype(np.float32)
        in_maps.append({
            "xP": xPb[b],
            "WqP": pack(WqT_i, 512),
            "WkP": pack(WkT_i, 128),
            "WvP": pack(WvT_i, 128),
            "WoP": pack(WoT_i, 512),
            **tables,
        })
    return in_maps


def kernel(hidden_states, cos, sin, Wq, Wk, Wv, Wo, _want_profile=False):
    from concourse.bass_utils import run_bass_kernel_spmd

    if "nc" not in _cache:
        _cache["nc"] = _build_graph()
    nc = _cache["nc"]
    in_maps = _host_prep(np.asarray(hidden_states), np.asarray(cos),
                         np.asarray(sin), np.asarray(Wq), np.asarray(Wk),
                         np.asarray(Wv), np.asarray(Wo))
    res = run_bass_kernel_spmd(nc, in_maps, list(range(N_CORES)),
                               trace=_want_profile)
    # assemble: core (b, r) holds out^T [512, L] = cols [512r, 512r+512) of b
    full = np.empty((B, L, HID), np.float32)
    for i in range(N_CORES):
        b, r = divmod(i, TP)
        full[b, :, CW * r:CW * (r + 1)] = res.results[i]["out"].T
    if _want_profile:
        return full, res
    return full


# revision 11
# speedup vs baseline: 1.3437x; 1.0843x over previous
"""Distributed GQA attention kernel for one TRN2 chip (8 NeuronCores).

Problem: B=2, L=2048, HID=2048, H=32 q-heads, HKV=8 kv-heads, D=64,
rotary embedding, causal softmax, o-proj.

Sharding: core i -> batch b=i//4, TP rank r=i%4.  Each core computes
8 q-heads / 2 kv-heads of its batch, all-gathers the attention outputs
(feature-major, bf16) within its 4-core TP group, then computes its
512 output columns of the o-proj.  Host assembles the full output.

Schedule (v3):
- proj(tt) and attention for head-pairs jj=0,1 interleave per q-tile,
  so the scalar-engine exp stream overlaps projection matmuls.
- AllGathers are split (h0: 2 token-halves, h1: token-half + 2 token-
  quarters) and fire as soon as their attention chunk completes; all
  collectives are emitted after the last DMA-transpose (xbar modes
  serialize against in-flight collectives).
- o-proj chunks are emitted where the PE has slack; the tail is one
  quarter-AG + 32 matmuls.
- All weight/x DRAM operands are host-packed so every load is one DMA
  with 128 contiguous per-partition descriptors.
- PSUM->SBUF copies, staging and normalization run on the vector
  engine; the scalar engine does exp (the attention pacer) only.

All matmuls run in bf16 with fp32 PSUM accumulation.  Softmax skips the
row-max (logits are bounded ~|6| for these input scales) and obtains
row sums for free by appending a 64-wide ones block to V's stationary
operand; normalization is a DVE reciprocal + multiply.
"""

import sys

sys.path.insert(0, "/opt/trn_rl_repo")

import numpy as np
import ml_dtypes

B, L, HID = 2, 2048, 2048
H, HKV, D = 32, 8, 64
N_CORES = 8
TP = 4           # tensor-parallel group size
HL = 8           # q heads per core
CW = 512         # o-proj output columns per core
TT = 4           # t tiles of 512 over L
CCH = HID // 128 # contraction chunks (16)
BF16 = ml_dtypes.bfloat16

_cache = {}


def _build_graph():
    import concourse.bass as bass
    import concourse.tile as tile
    from concourse import bacc, mybir

    dt = mybir.dt
    f32, bf16 = dt.float32, dt.bfloat16

    nc = bacc.Bacc("TRN2", target_bir_lowering=False, debug=False,
                   num_devices=N_CORES)

    # all packed [128, n] with per-partition-contiguous layout
    xP = nc.dram_tensor("xP", [128, TT * CCH * 512], bf16,
                        kind="ExternalInput")
    WqP = nc.dram_tensor("WqP", [128, CCH * 512], bf16, kind="ExternalInput")
    WkP = nc.dram_tensor("WkP", [128, CCH * 128], bf16, kind="ExternalInput")
    WvP = nc.dram_tensor("WvP", [128, CCH * 128], bf16, kind="ExternalInput")
    WoP = nc.dram_tensor("WoP", [128, CCH * 512], bf16, kind="ExternalInput")
    C1 = nc.dram_tensor("C1", [128, L], bf16, kind="ExternalInput")
    C2 = nc.dram_tensor("C2", [128, L], bf16, kind="ExternalInput")
    out = nc.dram_tensor("out", [CW, L], f32, kind="ExternalOutput")

    def bcast_m(ap2d, n):
        # [P, F] -> [P, n, F] with a step-0 middle dim (free-dim broadcast)
        return bass.AP(ap2d.tensor, ap2d.offset,
                       [ap2d.ap[0], [0, n], ap2d.ap[1]])

    with tile.TileContext(nc) as tc:
        with (
            tc.tile_pool(name="persist", bufs=1) as persist,
            tc.tile_pool(name="mm", bufs=2, space="PSUM") as pmm,
            tc.tile_pool(name="ps_s", bufs=2, space="PSUM") as ps_s,
            tc.tile_pool(name="po", bufs=2, space="PSUM") as po,
            tc.tile_pool(name="pp", bufs=4) as pp,
            tc.tile_pool(name="xt", bufs=2) as xtp,
            tc.tile_pool(name="rope", bufs=2) as rope,
            tc.tile_pool(name="aog", bufs=1) as aogp,
            tc.tile_pool(name="sta", bufs=16) as stap,
            tc.tile_pool(name="ost", bufs=2) as ostp,
            tc.tile_pool(name="dram", bufs=1, space="DRAM") as dram,
        ):
            # ---- persistent SBUF tensors ----
            qq = persist.tile([128, HL // 2 * L], bf16)      # roped Q^T, 2MB
            kk = persist.tile([128, L], bf16)                # roped K^T (2 kv)
            v2t = persist.tile([128, L], bf16)               # V^T staging
            v2 = persist.tile([128, CCH * 256], bf16)        # [V|1|V|1] per kt
            ao = persist.tile([128, HL // 2 * L], bf16)      # attn out^T
            c1 = persist.tile([128, L], bf16)
            c2 = persist.tile([128, L], bf16)
            wq_sb = persist.tile([128, CCH * 512], bf16)
            wk_sb = persist.tile([128, CCH * 128], bf16)
            wv_sb = persist.tile([128, CCH * 128], bf16)
            wo_sb = persist.tile([128, CCH * 512], bf16)
            warm = persist.tile([128, 256], bf16)

            # ---- warmup matmuls: keep the PE busy during initial DMA ----
            nc.gpsimd.memset(warm[:], 0.25)
            for i in range(28):
                psw = pmm.tile([128, 256], f32, tag="mm", name=f"warm{i}")
                nc.tensor.matmul(psw[:], lhsT=warm[:, 0:128], rhs=warm[:],
                                 start=True, stop=True)

            # ---- input DMAs (single contiguous load each) ----
            nc.sync.dma_start(wq_sb[:], WqP[:])
            xP_v = xP[:].rearrange("p (tt f) -> p tt f", f=CCH * 512)
            xts = []
            xt0 = xtp.tile([128, CCH * 512], bf16, tag="xt", name="xt0")
            nc.sync.dma_start(xt0[:], xP_v[:, 0])
            xts.append(xt0)
            nc.sync.dma_start(wk_sb[:], WkP[:])
            nc.sync.dma_start(wv_sb[:], WvP[:])
            nc.gpsimd.dma_start(c1[:], C1[:])
            nc.gpsimd.dma_start(c2[:], C2[:])
            nc.gpsimd.dma_start(wo_sb[:], WoP[:])

            # ones blocks of v2 (columns 64:128 and 192:256 of each kt group)
            for off in (64, 192):
                ones_view = bass.AP(v2.tensor, v2.offset + off,
                                    [v2.ap[0], [256, CCH], [1, 64]])
                nc.gpsimd.memset(ones_view, 1.0)

            # causal-mask helpers: ident for the mask matmul, bmask holds
            # -48 where q < 128*dj + k (dj = kt - 4*qT >= 0 diagonal block)
            ident = persist.tile([128, 128], bf16)
            nc.gpsimd.memset(ident[:], 1.0)
            nc.gpsimd.affine_select(
                out=ident[:], in_=ident[:], pattern=[[-1, 128]],
                compare_op=mybir.AluOpType.is_equal, fill=0.0,
                base=0, channel_multiplier=1)
            # triangle mask [128,128]: -48 where q' < k' (same for any
            # diagonal 128-block); off-triangle columns are simply skipped
            # by narrower S/PV matmuls.
            bmask = persist.tile([128, 128], bf16)
            nc.gpsimd.memset(bmask[:], -48.0)
            nc.gpsimd.affine_select(
                out=bmask[:], in_=bmask[:],
                pattern=[[-1, 128]], compare_op=mybir.AluOpType.is_gt,
                fill=0.0, base=0, channel_multiplier=1)

            def proj(tt):
                ts = slice(tt * 512, (tt + 1) * 512)
                xt = xts[tt]
                if tt + 1 < TT:  # prefetch next tile
                    xtn = xtp.tile([128, CCH * 512], bf16, tag="xt",
                                   name=f"xt{tt + 1}")
                    nc.sync.dma_start(xtn[:], xP_v[:, tt + 1])
                    xts.append(xtn)

                # --- Q: 4 M-tiles (head pair (jj, jj+4) each) ---
                qraw = rope.tile([128, 4 * 512], bf16, tag="qraw")
                for m in range(4):
                    psq = pmm.tile([128, 512], f32, tag="mm")
                    for c in range(CCH):
                        nc.tensor.matmul(
                            psq[:],
                            lhsT=wq_sb[:, c * 512 + m * 128:
                                       c * 512 + (m + 1) * 128],
                            rhs=xt[:, c * 512:(c + 1) * 512],
                            start=(c == 0), stop=(c == CCH - 1))
                    nc.vector.tensor_copy(qraw[:, m * 512:(m + 1) * 512],
                                          psq[:])

                # --- K ---
                kraw = rope.tile([128, 512], bf16, tag="kraw")
                psk = pmm.tile([128, 512], f32, tag="mm")
                for c in range(CCH):
                    nc.tensor.matmul(
                        psk[:], lhsT=wk_sb[:, c * 128:(c + 1) * 128],
                        rhs=xt[:, c * 512:(c + 1) * 512],
                        start=(c == 0), stop=(c == CCH - 1))
                nc.vector.tensor_copy(kraw[:], psk[:])

                # --- V ---
                psv = pmm.tile([128, 512], f32, tag="mm")
                for c in range(CCH):
                    nc.tensor.matmul(
                        psv[:], lhsT=wv_sb[:, c * 128:(c + 1) * 128],
                        rhs=xt[:, c * 512:(c + 1) * 512],
                        start=(c == 0), stop=(c == CCH - 1))
                nc.vector.tensor_copy(v2t[:, ts], psv[:])

                # --- RoPE on Q (in-place, no qtmp) ---
                qsw = rope.tile([128, 4 * 512], bf16, tag="qsw")
                for a, b_ in ((0, 32), (32, 0), (64, 96), (96, 64)):
                    nc.gpsimd.dma_start(qsw[b_:b_ + 32, :], qraw[a:a + 32, :])
                q3 = qraw[:].rearrange("p (m t) -> p m t", t=512)
                s3 = qsw[:].rearrange("p (m t) -> p m t", t=512)
                qqd = bass.AP(qq.tensor, qq.offset + tt * 512,
                              [qq.ap[0], [2048, 4], [1, 512]])
                nc.vector.tensor_tensor(qqd, q3, bcast_m(c1[:, ts], 4),
                                        mybir.AluOpType.mult)
                nc.vector.tensor_tensor(s3, s3, bcast_m(c2[:, ts], 4),
                                        mybir.AluOpType.mult)
                nc.vector.tensor_tensor(qqd, qqd, s3, mybir.AluOpType.add)

                # --- RoPE on K (in-place, no ktmp) ---
                ksw = rope.tile([128, 512], bf16, tag="ksw")
                for a, b_ in ((0, 32), (32, 0), (64, 96), (96, 64)):
                    nc.gpsimd.dma_start(ksw[b_:b_ + 32, :], kraw[a:a + 32, :])
                nc.vector.tensor_tensor(kraw[:], kraw[:], c1[:, ts],
                                        mybir.AluOpType.mult)
                nc.vector.tensor_tensor(ksw[:], ksw[:], c2[:, ts],
                                        mybir.AluOpType.mult)
                nc.vector.tensor_tensor(kk[:, ts], kraw[:], ksw[:],
                                        mybir.AluOpType.add)

                # --- V transpose to token-major (v2 cols 0:64/128:192) ---
                for g in range(2):
                    v2_dst = bass.AP(v2.tensor,
                                     v2.offset + (4 * tt) * 256 + g * 128,
                                     [v2.ap[0], [256, 4], [1, 64]])
                    nc.sync.dma_start_transpose(
                        v2_dst, v2t[g * 64:(g + 1) * 64, ts])

            def attn_group(jj, qT):
                """Attention for head pair (jj, jj+4), query tile qT."""
                nkt = 4 * qT + 4
                qoff = jj * L
                qs = slice(qoff + qT * 512, qoff + (qT + 1) * 512)
                o0 = po.tile([128, 512], f32, tag="o", name=f"o0_{jj}_{qT}")
                o1 = po.tile([128, 512], f32, tag="o", name=f"o1_{jj}_{qT}")
                for kp in range(nkt // 2):
                    sb0 = ps_s.tile([128, 1024], f32, tag="s",
                                    name=f"sb0_{jj}_{qT}_{kp}")
                    sb1 = ps_s.tile([128, 1024], f32, tag="s",
                                    name=f"sb1_{jj}_{qT}_{kp}")
                    for h in range(2):
                        kt = 2 * kp + h
                        ksl = slice(kt * 128, (kt + 1) * 128)
                        hs = slice(h * 512, (h + 1) * 512)
                        dj = kt - 4 * qT  # >=0 -> diagonal block
                        # diagonal blocks: only columns q >= 128*dj can be
                        # unmasked; compute the narrower [128*dj, 512) range
                        # and add the -48 triangle on its first 128 columns.
                        cut = 128 * dj if dj >= 0 else 0
                        nc.tensor.matmul(
                            sb0[:, hs.start + cut:hs.stop],
                            lhsT=kk[0:64, ksl],
                            rhs=qq[0:64, qs.start + cut:qs.stop], start=True,
                            stop=(dj < 0), tile_position=(0, 0))
                        nc.tensor.matmul(
                            sb1[:, hs.start + cut:hs.stop],
                            lhsT=kk[64:128, ksl],
                            rhs=qq[64:128, qs.start + cut:qs.stop],
                            start=True,
                            stop=(dj < 0), tile_position=(64, 0))
                        if dj >= 0:
                            nc.tensor.matmul(
                                sb0[:, hs.start + cut:hs.start + cut + 128],
                                lhsT=ident[:], rhs=bmask[:],
                                start=False, stop=True)
                            nc.tensor.matmul(
                                sb1[:, hs.start + cut:hs.start + cut + 128],
                                lhsT=ident[:], rhs=bmask[:],
                                start=False, stop=True)
                    p0 = pp.tile([128, 1024], bf16, tag="p",
                                 name=f"p0_{jj}_{qT}_{kp}")
                    p1 = pp.tile([128, 1024], bf16, tag="p",
                                 name=f"p1_{jj}_{qT}_{kp}")
                    nc.scalar.activation(
                        p0[:], sb0[:], mybir.ActivationFunctionType.Exp)
                    nc.scalar.activation(
                        p1[:], sb1[:], mybir.ActivationFunctionType.Exp)
                    for h in range(2):
                        kt = 2 * kp + h
                        hs = slice(h * 512, (h + 1) * 512)
                        dj = kt - 4 * qT
                        cut = 128 * dj if dj > 0 else 0
                        nc.tensor.matmul(
                            o0[:, cut:512],
                            lhsT=v2[:, kt * 256:kt * 256 + 128],
                            rhs=p0[:, hs.start + cut:hs.stop],
                            start=(kt == 0), stop=(kt == nkt - 1))
                        nc.tensor.matmul(
                            o1[:, cut:512],
                            lhsT=v2[:, kt * 256 + 128:(kt + 1) * 256],
                            rhs=p1[:, hs.start + cut:hs.stop],
                            start=(kt == 0), stop=(kt == nkt - 1))
                # normalize (approx-recip full tile; rows 64:128 hold the
                # replicated sums - base!=0 slices break the custom-DVE op)
                rc = pp.tile([128, 512], f32, tag="rc", bufs=2,
                             name=f"rc_{jj}_{qT}")
                nc.vector.reciprocal_approx_fast(rc[:], o0[:])
                nc.vector.tensor_tensor(
                    ao[0:64, qs], o0[0:64, :], rc[64:128, :],
                    mybir.AluOpType.mult)
                rc2 = pp.tile([128, 512], f32, tag="rc", bufs=2,
                              name=f"rc2_{jj}_{qT}")
                nc.vector.reciprocal_approx_fast(rc2[:], o1[:])
                nc.vector.tensor_tensor(
                    ao[64:128, qs], o1[0:64, :], rc2[64:128, :],
                    mybir.AluOpType.mult)

            # AG pieces: (head-half hh, token range [t0, t1) in 512-tiles)
            bounces = {}
            gaths = {}
            pieces = [(0, 0, 2), (0, 2, 4), (1, 0, 2), (1, 2, 3), (1, 3, 4)]
            for hh, t0, t1 in pieces:
                w = (t1 - t0) * 512
                bounces[(hh, t0)] = dram.tile([2 * 128, w], bf16,
                                              name=f"bounce{hh}_{t0}")
                gaths[(hh, t0)] = dram.tile([TP * 2 * 128, w], bf16,
                                            name=f"gath{hh}_{t0}")

            def ship(hh, t0, t1):
                """Bounce ao (head pair hh, token tiles [t0,t1)) + AG."""
                bnc = bounces[(hh, t0)]
                w = (t1 - t0) * 512
                for g in range(2):
                    for jj in (2 * hh, 2 * hh + 1):
                        r0 = 128 * g + 64 * (jj - 2 * hh)
                        nc.sync.dma_start(
                            bnc[r0:r0 + 64, :],
                            ao[g * 64:(g + 1) * 64,
                               jj * L + t0 * 512:jj * L + t1 * 512])
                nc.gpsimd.collective_compute(
                    "AllGather", mybir.AluOpType.bypass,
                    replica_groups=[[0, 1, 2, 3], [4, 5, 6, 7]],
                    ins=[bnc.opt()], outs=[gaths[(hh, t0)].opt()])

            stas = {}

            def oproj(hh, t0, t1):
                """o-proj partial for gather piece (hh, [t0,t1))."""
                w = (t1 - t0) * 512
                aok = aogp.tile([128, 8 * 1024], bf16, tag="aok",
                                name=f"aok{hh}_{t0}")
                av = bass.AP(aok.tensor, aok.offset,
                             [aok.ap[0], [w, 8], [1, w]])
                nc.sync.dma_start(
                    av, gaths[(hh, t0)][:].rearrange("(c p) t -> p c t",
                                                     p=128))
                for tt in range(t0, t1):
                    tl = tt - t0
                    ts = slice(tt * 512, (tt + 1) * 512)
                    for ct in range(4):
                        pso = pmm.tile([128, 512], f32, tag="mm",
                                       name=f"pso{hh}_{tt}_{ct}")
                        for c in range(8):
                            nc.tensor.matmul(
                                pso[:],
                                lhsT=wo_sb[:, (8 * hh + c) * 512 + ct * 128:
                                           (8 * hh + c) * 512 +
                                           (ct + 1) * 128],
                                rhs=aok[:, c * w + tl * 512:
                                        c * w + (tl + 1) * 512],
                                start=(c == 0), stop=(c == 7))
                        if hh == 0:
                            sta = stap.tile([128, 512], bf16, tag="sta",
                                            name=f"sta{tt}_{ct}")
                            nc.vector.tensor_copy(sta[:], pso[:])
                            stas[(tt, ct)] = sta
                        else:
                            ost = ostp.tile([128, 512], f32, tag="ost",
                                            name=f"ost{tt}_{ct}")
                            nc.vector.tensor_tensor(
                                ost[:], pso[:], stas[(tt, ct)][:],
                                mybir.AluOpType.add)
                            nc.scalar.dma_start(
                                out[ct * 128:(ct + 1) * 128, ts], ost[:])

            # ================= schedule =================
            for tt in range(TT):
                proj(tt)
                attn_group(0, tt)
                attn_group(1, tt)
            # all DMA-transposes precede the first collective (xbar modes
            # serialize against in-flight collective SDMA traffic)
            ship(0, 0, 2)
            ship(0, 2, 4)

            attn_group(2, 0)
            attn_group(3, 0)
            attn_group(2, 1)
            attn_group(3, 1)
            oproj(0, 0, 2)
            ship(1, 0, 2)

            attn_group(2, 2)
            attn_group(3, 2)
            oproj(0, 2, 4)
            ship(1, 2, 3)

            attn_group(2, 3)
            attn_group(3, 3)
            oproj(1, 0, 2)
            ship(1, 3, 4)

            oproj(1, 2, 3)
            oproj(1, 3, 4)

    nc.compile()
    return nc


def _host_prep(hidden_states, cos, sin, Wq, Wk, Wv, Wo):
    """Build the 8 per-core input maps (all host-side packing)."""
    scale = float(D) ** -0.5
    # rope coefficient tables [128, L]: 4 groups of 32 rows (d 0:32 pattern)
    cosT = cos[:, :32].T.astype(np.float32)          # [32, L]
    sinT = sin[:, :32].T.astype(np.float32)
    c1 = np.tile(cosT, (4, 1))                       # [128, L]
    c2 = np.concatenate([-sinT, sinT, -sinT, sinT], axis=0)
    tables = {"C1": c1.astype(BF16), "C2": c2.astype(BF16)}

    def pack(WT, m):
        # WT [HID, m] -> [128, CCH*m] with row p = concat_c WT[c*128+p, :]
        return np.ascontiguousarray(
            WT.reshape(CCH, 128, m).transpose(1, 0, 2).reshape(128, CCH * m)
        ).astype(BF16)

    # x packed per (tt, c): [128, tt, c, 512]
    xPb = []
    for b in range(B):
        xT = hidden_states[b].T.astype(np.float32)   # [HID, L]
        xp = (xT.reshape(CCH, 128, TT, 512).transpose(1, 2, 0, 3)
              .reshape(128, TT * CCH * 512))
        xPb.append(np.ascontiguousarray(xp).astype(BF16))

    in_maps = []
    for i in range(N_CORES):
        b, r = divmod(i, TP)
        # Wq rows reordered: M-tile m = heads (8r+m, 8r+4+m); scale folded in
        rows = []
        for m in range(4):
            rows.append(Wq[(8 * r + m) * D:(8 * r + m + 1) * D])
            rows.append(Wq[(8 * r + 4 + m) * D:(8 * r + 4 + m + 1) * D])
        WqT_i = (np.concatenate(rows, 0) * scale).T.astype(np.float32)
        WkT_i = Wk[2 * r * D:(2 * r + 2) * D].T.astype(np.float32)
        WvT_i = Wv[2 * r * D:(2 * r + 2) * D].T.astype(np.float32)
        # o-proj k-rows ordered to match the gathered buffers:
        # half h row R: rank=R//256, g=(R%256)//128, jj=2h+(R%128)//64, d=R%64
        RR = np.arange(1024)
        perm = []
        for h in range(2):
            f = ((8 * (RR // 256) + 4 * ((RR % 256) // 128)
                  + 2 * h + (RR % 128) // 64) * D + RR % 64)
            perm.append(f)
        perm = np.concatenate(perm)
        WoT_i = Wo[CW * r:CW * (r + 1), :].T[perm].astype(np.float32)
        in_maps.append({
            "xP": xPb[b],
            "WqP": pack(WqT_i, 512),
            "WkP": pack(WkT_i, 128),
            "WvP": pack(WvT_i, 128),
            "WoP": pack(WoT_i, 512),
            **tables,
        })
    return in_maps


def kernel(hidden_states, cos, sin, Wq, Wk, Wv, Wo, _want_profile=False):
    from concourse.bass_utils import run_bass_kernel_spmd

    if "nc" not in _cache:
        _cache["nc"] = _build_graph()
    nc = _cache["nc"]
    in_maps = _host_prep(np.asarray(hidden_states), np.asarray(cos),
                         np.asarray(sin), np.asarray(Wq), np.asarray(Wk),
                         np.asarray(Wv), np.asarray(Wo))
    res = run_bass_kernel_spmd(nc, in_maps, list(range(N_CORES)),
                               trace=_want_profile)
    # assemble: core (b, r) holds out^T [512, L] = cols [512r, 512r+512) of b
    full = np.empty((B, L, HID), np.float32)
    for i in range(N_CORES):
        b, r = divmod(i, TP)
        full[b, :, CW * r:CW * (r + 1)] = res.results[i]["out"].T
    if _want_profile:
        return full, res
    return full


# revision 18
# speedup vs baseline: 1.3695x; 1.0192x over previous
"""Distributed GQA attention kernel for one TRN2 chip (8 NeuronCores).

Problem: B=2, L=2048, HID=2048, H=32 q-heads, HKV=8 kv-heads, D=64,
rotary embedding, causal softmax, o-proj.

Sharding: core i -> batch b=i//4, TP rank r=i%4.  Each core computes
8 q-heads / 2 kv-heads of its batch, all-gathers the attention outputs
(feature-major, bf16) within its 4-core TP group, then computes its
512 output columns of the o-proj.  Host assembles the full output.

Schedule (v3):
- proj(tt) and attention for head-pairs jj=0,1 interleave per q-tile,
  so the scalar-engine exp stream overlaps projection matmuls.
- AllGathers are split (h0: 2 token-halves, h1: token-half + 2 token-
  quarters) and fire as soon as their attention chunk completes; all
  collectives are emitted after the last DMA-transpose (xbar modes
  serialize against in-flight collectives).
- o-proj chunks are emitted where the PE has slack; the tail is one
  quarter-AG + 32 matmuls.
- All weight/x DRAM operands are host-packed so every load is one DMA
  with 128 contiguous per-partition descriptors.
- PSUM->SBUF copies, staging and normalization run on the vector
  engine; the scalar engine does exp (the attention pacer) only.

All matmuls run in bf16 with fp32 PSUM accumulation.  Softmax skips the
row-max (logits are bounded ~|6| for these input scales) and obtains
row sums for free by appending a 64-wide ones block to V's stationary
operand; normalization is a DVE reciprocal + multiply.
"""

import sys

sys.path.insert(0, "/opt/trn_rl_repo")

import numpy as np
import ml_dtypes

B, L, HID = 2, 2048, 2048
H, HKV, D = 32, 8, 64
N_CORES = 8
TP = 4           # tensor-parallel group size
HL = 8           # q heads per core
CW = 512         # o-proj output columns per core
TT = 4           # t tiles of 512 over L
CCH = HID // 128 # contraction chunks (16)
BF16 = ml_dtypes.bfloat16

_cache = {}


def _build_graph():
    import concourse.bass as bass
    import concourse.tile as tile
    from concourse import bacc, mybir

    dt = mybir.dt
    f32, bf16 = dt.float32, dt.bfloat16

    nc = bacc.Bacc("TRN2", target_bir_lowering=False, debug=False,
                   num_devices=N_CORES)

    # all packed [128, n] with per-partition-contiguous layout
    xP = nc.dram_tensor("xP", [128, TT * CCH * 512], bf16,
                        kind="ExternalInput")
    WqP = nc.dram_tensor("WqP", [128, CCH * 512], bf16, kind="ExternalInput")
    WkP = nc.dram_tensor("WkP", [128, CCH * 128], bf16, kind="ExternalInput")
    WvP = nc.dram_tensor("WvP", [128, CCH * 128], bf16, kind="ExternalInput")
    WoP = nc.dram_tensor("WoP", [128, CCH * 512], bf16, kind="ExternalInput")
    C1 = nc.dram_tensor("C1", [128, L], bf16, kind="ExternalInput")
    C2 = nc.dram_tensor("C2", [128, L], bf16, kind="ExternalInput")
    out = nc.dram_tensor("out", [CW, L], f32, kind="ExternalOutput")

    def bcast_m(ap2d, n):
        # [P, F] -> [P, n, F] with a step-0 middle dim (free-dim broadcast)
        return bass.AP(ap2d.tensor, ap2d.offset,
                       [ap2d.ap[0], [0, n], ap2d.ap[1]])

    with tile.TileContext(nc) as tc:
        with (
            tc.tile_pool(name="persist", bufs=1) as persist,
            tc.tile_pool(name="mm", bufs=2, space="PSUM") as pmm,
            tc.tile_pool(name="ps_s", bufs=2, space="PSUM") as ps_s,
            tc.tile_pool(name="po", bufs=2, space="PSUM") as po,
            tc.tile_pool(name="pp", bufs=4) as pp,
            tc.tile_pool(name="xt", bufs=2) as xtp,
            tc.tile_pool(name="rope", bufs=2) as rope,
            tc.tile_pool(name="aog", bufs=1) as aogp,
            tc.tile_pool(name="sta", bufs=16) as stap,
            tc.tile_pool(name="ost", bufs=2) as ostp,
            tc.tile_pool(name="dram", bufs=1, space="DRAM") as dram,
        ):
            # ---- persistent SBUF tensors ----
            qq = persist.tile([128, HL // 2 * L], bf16)      # roped Q^T, 2MB
            kk = persist.tile([128, L], bf16)                # roped K^T (2 kv)
            v2t = persist.tile([128, L], bf16)               # V^T staging
            v2 = persist.tile([128, CCH * 256], bf16)        # [V|1|V|1] per kt
            ao = persist.tile([128, HL // 2 * L], bf16)      # attn out^T
            c1 = persist.tile([128, L], bf16)
            c2 = persist.tile([128, L], bf16)
            wq_sb = persist.tile([128, CCH * 512], bf16)
            wk_sb = persist.tile([128, CCH * 128], bf16)
            wv_sb = persist.tile([128, CCH * 128], bf16)
            wo_sb = persist.tile([128, CCH * 512], bf16)
            warm = persist.tile([128, 256], bf16)

            # ---- warmup matmuls: keep the PE busy during initial DMA ----
            nc.gpsimd.memset(warm[:], 0.25)
            for i in range(45):
                psw = pmm.tile([128, 256], f32, tag="mm", name=f"warm{i}")
                nc.tensor.matmul(psw[:], lhsT=warm[:, 0:128], rhs=warm[:],
                                 start=True, stop=True)

            # ---- input DMAs (single contiguous load each) ----
            nc.sync.dma_start(wq_sb[:], WqP[:])
            xP_v = xP[:].rearrange("p (tt f) -> p tt f", f=CCH * 512)
            xts = []
            xt0 = xtp.tile([128, CCH * 512], bf16, tag="xt", name="xt0")
            nc.sync.dma_start(xt0[:], xP_v[:, 0])
            xts.append(xt0)
            nc.sync.dma_start(wk_sb[:], WkP[:])
            nc.sync.dma_start(wv_sb[:], WvP[:])
            nc.gpsimd.dma_start(c1[:], C1[:])
            nc.gpsimd.dma_start(c2[:], C2[:])
            nc.gpsimd.dma_start(wo_sb[:], WoP[:])

            # ones blocks of v2 (columns 64:128 and 192:256 of each kt group)
            for off in (64, 192):
                ones_view = bass.AP(v2.tensor, v2.offset + off,
                                    [v2.ap[0], [256, CCH], [1, 64]])
                nc.gpsimd.memset(ones_view, 1.0)

            # causal-mask helpers: ident for the mask matmul, bmask holds
            # -48 where q < 128*dj + k (dj = kt - 4*qT >= 0 diagonal block)
            ident = persist.tile([128, 128], bf16)
            nc.gpsimd.memset(ident[:], 1.0)
            nc.gpsimd.affine_select(
                out=ident[:], in_=ident[:], pattern=[[-1, 128]],
                compare_op=mybir.AluOpType.is_equal, fill=0.0,
                base=0, channel_multiplier=1)
            # triangle mask [128,128]: -48 where q' < k' (same for any
            # diagonal 128-block); off-triangle columns are simply skipped
            # by narrower S/PV matmuls.
            bmask = persist.tile([128, 128], bf16)
            nc.gpsimd.memset(bmask[:], -48.0)
            nc.gpsimd.affine_select(
                out=bmask[:], in_=bmask[:],
                pattern=[[-1, 128]], compare_op=mybir.AluOpType.is_gt,
                fill=0.0, base=0, channel_multiplier=1)

            # dummy first collective: the first AG on the CC path costs
            # ~25us extra; absorb it here, overlapped with the projections
            bounce_d = dram.tile([128, 64], bf16, name="bounce_d")
            gath_d = dram.tile([TP * 128, 64], bf16, name="gath_d")
            nc.sync.dma_start(bounce_d[:], warm[:, 0:64])
            nc.gpsimd.collective_compute(
                "AllGather", mybir.AluOpType.bypass,
                replica_groups=[[0, 1, 2, 3], [4, 5, 6, 7]],
                ins=[bounce_d.opt()], outs=[gath_d.opt()])

            def proj(tt):
                ts = slice(tt * 512, (tt + 1) * 512)
                xt = xts[tt]
                if tt + 1 < TT:  # prefetch next tile
                    xtn = xtp.tile([128, CCH * 512], bf16, tag="xt",
                                   name=f"xt{tt + 1}")
                    nc.sync.dma_start(xtn[:], xP_v[:, tt + 1])
                    xts.append(xtn)

                # --- Q: 4 M-tiles (head pair (jj, jj+4) each) ---
                qraw = rope.tile([128, 4 * 512], bf16, tag="qraw")
                for m in range(4):
                    psq = pmm.tile([128, 512], f32, tag="mm")
                    for c in range(CCH):
                        nc.tensor.matmul(
                            psq[:],
                            lhsT=wq_sb[:, c * 512 + m * 128:
                                       c * 512 + (m + 1) * 128],
                            rhs=xt[:, c * 512:(c + 1) * 512],
                            start=(c == 0), stop=(c == CCH - 1))
                    nc.vector.tensor_copy(qraw[:, m * 512:(m + 1) * 512],
                                          psq[:])

                # --- K ---
                kraw = rope.tile([128, 512], bf16, tag="kraw")
                psk = pmm.tile([128, 512], f32, tag="mm")
                for c in range(CCH):
                    nc.tensor.matmul(
                        psk[:], lhsT=wk_sb[:, c * 128:(c + 1) * 128],
                        rhs=xt[:, c * 512:(c + 1) * 512],
                        start=(c == 0), stop=(c == CCH - 1))
                nc.vector.tensor_copy(kraw[:], psk[:])

                # --- V ---
                psv = pmm.tile([128, 512], f32, tag="mm")
                for c in range(CCH):
                    nc.tensor.matmul(
                        psv[:], lhsT=wv_sb[:, c * 128:(c + 1) * 128],
                        rhs=xt[:, c * 512:(c + 1) * 512],
                        start=(c == 0), stop=(c == CCH - 1))
                nc.vector.tensor_copy(v2t[:, ts], psv[:])

                # --- RoPE on Q (in-place, no qtmp) ---
                qsw = rope.tile([128, 4 * 512], bf16, tag="qsw")
                for a, b_ in ((0, 32), (32, 0), (64, 96), (96, 64)):
                    nc.gpsimd.dma_start(qsw[b_:b_ + 32, :], qraw[a:a + 32, :])
                q3 = qraw[:].rearrange("p (m t) -> p m t", t=512)
                s3 = qsw[:].rearrange("p (m t) -> p m t", t=512)
                qqd = bass.AP(qq.tensor, qq.offset + tt * 512,
                              [qq.ap[0], [2048, 4], [1, 512]])
                nc.vector.tensor_tensor(qqd, q3, bcast_m(c1[:, ts], 4),
                                        mybir.AluOpType.mult)
                nc.vector.tensor_tensor(s3, s3, bcast_m(c2[:, ts], 4),
                                        mybir.AluOpType.mult)
                nc.vector.tensor_tensor(qqd, qqd, s3, mybir.AluOpType.add)

                # --- RoPE on K (in-place, no ktmp) ---
                ksw = rope.tile([128, 512], bf16, tag="ksw")
                for a, b_ in ((0, 32), (32, 0), (64, 96), (96, 64)):
                    nc.gpsimd.dma_start(ksw[b_:b_ + 32, :], kraw[a:a + 32, :])
                nc.vector.tensor_tensor(kraw[:], kraw[:], c1[:, ts],
                                        mybir.AluOpType.mult)
                nc.vector.tensor_tensor(ksw[:], ksw[:], c2[:, ts],
                                        mybir.AluOpType.mult)
                nc.vector.tensor_tensor(kk[:, ts], kraw[:], ksw[:],
                                        mybir.AluOpType.add)

                # --- V transpose to token-major via PE transposes (DMA
                # transposes would serialize against collective SDMA) ---
                for g in range(2):
                    pst = pmm.tile([128, 256], bf16, tag="mm",
                                   name=f"vt{tt}_{g}")
                    for q in range(4):
                        nc.tensor.transpose(
                            pst[:, q * 64:(q + 1) * 64],
                            v2t[g * 64:(g + 1) * 64,
                                tt * 512 + q * 128:tt * 512 + (q + 1) * 128],
                            ident[g * 64:(g + 1) * 64, g * 64:(g + 1) * 64])
                    v2_dst = bass.AP(v2.tensor,
                                     v2.offset + (4 * tt) * 256 + g * 128,
                                     [v2.ap[0], [256, 4], [1, 64]])
                    nc.vector.tensor_copy(
                        v2_dst, pst[:].rearrange("p (q d) -> p q d", d=64))

            def attn_group(jj, qT):
                """Attention for head pair (jj, jj+4), query tile qT."""
                nkt = 4 * qT + 4
                qoff = jj * L
                qs = slice(qoff + qT * 512, qoff + (qT + 1) * 512)
                o0 = po.tile([128, 512], f32, tag="o", name=f"o0_{jj}_{qT}")
                o1 = po.tile([128, 512], f32, tag="o", name=f"o1_{jj}_{qT}")
                for kp in range(nkt // 2):
                    sb0 = ps_s.tile([128, 1024], f32, tag="s",
                                    name=f"sb0_{jj}_{qT}_{kp}")
                    sb1 = ps_s.tile([128, 1024], f32, tag="s",
                                    name=f"sb1_{jj}_{qT}_{kp}")
                    for h in range(2):
                        kt = 2 * kp + h
                        ksl = slice(kt * 128, (kt + 1) * 128)
                        hs = slice(h * 512, (h + 1) * 512)
                        dj = kt - 4 * qT  # >=0 -> diagonal block
                        # diagonal blocks: only columns q >= 128*dj can be
                        # unmasked; compute the narrower [128*dj, 512) range
                        # and add the -48 triangle on its first 128 columns.
                        cut = 128 * dj if dj >= 0 else 0
                        nc.tensor.matmul(
                            sb0[:, hs.start + cut:hs.stop],
                            lhsT=kk[0:64, ksl],
                            rhs=qq[0:64, qs.start + cut:qs.stop], start=True,
                            stop=(dj < 0), tile_position=(0, 0))
                        nc.tensor.matmul(
                            sb1[:, hs.start + cut:hs.stop],
                            lhsT=kk[64:128, ksl],
                            rhs=qq[64:128, qs.start + cut:qs.stop],
                            start=True,
                            stop=(dj < 0), tile_position=(64, 0))
                        if dj >= 0:
                            nc.tensor.matmul(
                                sb0[:, hs.start + cut:hs.start + cut + 128],
                                lhsT=ident[:], rhs=bmask[:],
                                start=False, stop=True)
                            nc.tensor.matmul(
                                sb1[:, hs.start + cut:hs.start + cut + 128],
                                lhsT=ident[:], rhs=bmask[:],
                                start=False, stop=True)
                    p0 = pp.tile([128, 1024], bf16, tag="p",
                                 name=f"p0_{jj}_{qT}_{kp}")
                    p1 = pp.tile([128, 1024], bf16, tag="p",
                                 name=f"p1_{jj}_{qT}_{kp}")
                    nc.scalar.activation(
                        p0[:], sb0[:], mybir.ActivationFunctionType.Exp)
                    nc.scalar.activation(
                        p1[:], sb1[:], mybir.ActivationFunctionType.Exp)
                    for h in range(2):
                        kt = 2 * kp + h
                        hs = slice(h * 512, (h + 1) * 512)
                        dj = kt - 4 * qT
                        cut = 128 * dj if dj > 0 else 0
                        nc.tensor.matmul(
                            o0[:, cut:512],
                            lhsT=v2[:, kt * 256:kt * 256 + 128],
                            rhs=p0[:, hs.start + cut:hs.stop],
                            start=(kt == 0), stop=(kt == nkt - 1))
                        nc.tensor.matmul(
                            o1[:, cut:512],
                            lhsT=v2[:, kt * 256 + 128:(kt + 1) * 256],
                            rhs=p1[:, hs.start + cut:hs.stop],
                            start=(kt == 0), stop=(kt == nkt - 1))
                # normalize (approx-recip full tile; rows 64:128 hold the
                # replicated sums - base!=0 slices break the custom-DVE op)
                rc = pp.tile([128, 512], f32, tag="rc", bufs=2,
                             name=f"rc_{jj}_{qT}")
                nc.vector.reciprocal_approx_fast(rc[:], o0[:])
                nc.vector.tensor_tensor(
                    ao[0:64, qs], o0[0:64, :], rc[64:128, :],
                    mybir.AluOpType.mult)
                rc2 = pp.tile([128, 512], f32, tag="rc", bufs=2,
                              name=f"rc2_{jj}_{qT}")
                nc.vector.reciprocal_approx_fast(rc2[:], o1[:])
                nc.vector.tensor_tensor(
                    ao[64:128, qs], o1[0:64, :], rc2[64:128, :],
                    mybir.AluOpType.mult)

            # AG pieces: (head-half hh, token range [t0, t1) in 512-tiles)
            bounces = {}
            gaths = {}
            pieces = [(0, 0, 2), (0, 2, 4), (1, 0, 2), (1, 2, 3), (1, 3, 4)]
            for hh, t0, t1 in pieces:
                w = (t1 - t0) * 512
                bounces[(hh, t0)] = dram.tile([2 * 128, w], bf16,
                                              name=f"bounce{hh}_{t0}")
                gaths[(hh, t0)] = dram.tile([TP * 2 * 128, w], bf16,
                                            name=f"gath{hh}_{t0}")

            def ship(hh, t0, t1):
                """Bounce ao (head pair hh, token tiles [t0,t1)) + AG."""
                bnc = bounces[(hh, t0)]
                w = (t1 - t0) * 512
                for g in range(2):
                    for jj in (2 * hh, 2 * hh + 1):
                        r0 = 128 * g + 64 * (jj - 2 * hh)
                        nc.sync.dma_start(
                            bnc[r0:r0 + 64, :],
                            ao[g * 64:(g + 1) * 64,
                               jj * L + t0 * 512:jj * L + t1 * 512])
                nc.gpsimd.collective_compute(
                    "AllGather", mybir.AluOpType.bypass,
                    replica_groups=[[0, 1, 2, 3], [4, 5, 6, 7]],
                    ins=[bnc.opt()], outs=[gaths[(hh, t0)].opt()])

            stas = {}

            def oproj(hh, t0, t1):
                """o-proj partial for gather piece (hh, [t0,t1))."""
                w = (t1 - t0) * 512
                aok = aogp.tile([128, 8 * 1024], bf16, tag="aok",
                                name=f"aok{hh}_{t0}")
                av = bass.AP(aok.tensor, aok.offset,
                             [aok.ap[0], [w, 8], [1, w]])
                nc.sync.dma_start(
                    av, gaths[(hh, t0)][:].rearrange("(c p) t -> p c t",
                                                     p=128))
                for tt in range(t0, t1):
                    tl = tt - t0
                    ts = slice(tt * 512, (tt + 1) * 512)
                    for ct in range(4):
                        pso = pmm.tile([128, 512], f32, tag="mm",
                                       name=f"pso{hh}_{tt}_{ct}")
                        for c in range(8):
                            nc.tensor.matmul(
                                pso[:],
                                lhsT=wo_sb[:, (8 * hh + c) * 512 + ct * 128:
                                           (8 * hh + c) * 512 +
                                           (ct + 1) * 128],
                                rhs=aok[:, c * w + tl * 512:
                                        c * w + (tl + 1) * 512],
                                start=(c == 0), stop=(c == 7))
                        if hh == 0:
                            sta = stap.tile([128, 512], bf16, tag="sta",
                                            name=f"sta{tt}_{ct}")
                            nc.vector.tensor_copy(sta[:], pso[:])
                            stas[(tt, ct)] = sta
                        else:
                            ost = ostp.tile([128, 512], f32, tag="ost",
                                            name=f"ost{tt}_{ct}")
                            nc.vector.tensor_tensor(
                                ost[:], pso[:], stas[(tt, ct)][:],
                                mybir.AluOpType.add)
                            nc.scalar.dma_start(
                                out[ct * 128:(ct + 1) * 128, ts], ost[:])

            # ================= schedule =================
            for tt in range(TT):
                proj(tt)
                attn_group(0, tt)
                attn_group(1, tt)
                if tt == 1:
                    ship(0, 0, 2)
            ship(0, 2, 4)

            attn_group(2, 3)
            attn_group(3, 3)
            oproj(0, 0, 2)
            ship(1, 3, 4)

            attn_group(2, 0)
            attn_group(3, 0)
            attn_group(2, 1)
            attn_group(3, 1)
            oproj(0, 2, 4)
            ship(1, 0, 2)

            attn_group(2, 2)
            attn_group(3, 2)
            oproj(1, 3, 4)
            ship(1, 2, 3)

            oproj(1, 0, 2)
            oproj(1, 2, 3)

    nc.compile()
    return nc


def _host_prep(hidden_states, cos, sin, Wq, Wk, Wv, Wo):
    """Build the 8 per-core input maps (all host-side packing)."""
    scale = float(D) ** -0.5
    # rope coefficient tables [128, L]: 4 groups of 32 rows (d 0:32 pattern)
    cosT = cos[:, :32].T.astype(np.float32)          # [32, L]
    sinT = sin[:, :32].T.astype(np.float32)
    c1 = np.tile(cosT, (4, 1))                       # [128, L]
    c2 = np.concatenate([-sinT, sinT, -sinT, sinT], axis=0)
    tables = {"C1": c1.astype(BF16), "C2": c2.astype(BF16)}

    def pack(WT, m):
        # WT [HID, m] -> [128, CCH*m] with row p = concat_c WT[c*128+p, :]
        return np.ascontiguousarray(
            WT.reshape(CCH, 128, m).transpose(1, 0, 2).reshape(128, CCH * m)
        ).astype(BF16)

    # x packed per (tt, c): [128, tt, c, 512]
    xPb = []
    for b in range(B):
        xT = hidden_states[b].T.astype(np.float32)   # [HID, L]
        xp = (xT.reshape(CCH, 128, TT, 512).transpose(1, 2, 0, 3)
              .reshape(128, TT * CCH * 512))
        xPb.append(np.ascontiguousarray(xp).astype(BF16))

    in_maps = []
    for i in range(N_CORES):
        b, r = divmod(i, TP)
        # Wq rows reordered: M-tile m = heads (8r+m, 8r+4+m); scale folded in
        rows = []
        for m in range(4):
            rows.append(Wq[(8 * r + m) * D:(8 * r + m + 1) * D])
            rows.append(Wq[(8 * r + 4 + m) * D:(8 * r + 4 + m + 1) * D])
        WqT_i = (np.concatenate(rows, 0) * scale).T.astype(np.float32)
        WkT_i = Wk[2 * r * D:(2 * r + 2) * D].T.astype(np.float32)
        WvT_i = Wv[2 * r * D:(2 * r + 2) * D].T.astype(np.float32)
        # o-proj k-rows ordered to match the gathered buffers:
        # half h row R: rank=R//256, g=(R%256)//128, jj=2h+(R%128)//64, d=R%64
        RR = np.arange(1024)
        perm = []
        for h in range(2):
            f = ((8 * (RR // 256) + 4 * ((RR % 256) // 128)
                  + 2 * h + (RR % 128) // 64) * D + RR % 64)
            perm.append(f)
        perm = np.concatenate(perm)
        WoT_i = Wo[CW * r:CW * (r + 1), :].T[perm].astype(np.float32)
        in_maps.append({
            "xP": xPb[b],
            "WqP": pack(WqT_i, 512),
            "WkP": pack(WkT_i, 128),
            "WvP": pack(WvT_i, 128),
            "WoP": pack(WoT_i, 512),
            **tables,
        })
    return in_maps


def kernel(hidden_states, cos, sin, Wq, Wk, Wv, Wo, _want_profile=False):
    from concourse.bass_utils import run_bass_kernel_spmd

    if "nc" not in _cache:
        _cache["nc"] = _build_graph()
    nc = _cache["nc"]
    in_maps = _host_prep(np.asarray(hidden_states), np.asarray(cos),
                         np.asarray(sin), np.asarray(Wq), np.asarray(Wk),
                         np.asarray(Wv), np.asarray(Wo))
    res = run_bass_kernel_spmd(nc, in_maps, list(range(N_CORES)),
                               trace=_want_profile)
    # assemble: core (b, r) holds out^T [512, L] = cols [512r, 512r+512) of b
    full = np.empty((B, L, HID), np.float32)
    for i in range(N_CORES):
        b, r = divmod(i, TP)
        full[b, :, CW * r:CW * (r + 1)] = res.results[i]["out"].T
    if _want_profile:
        return full, res
    return full
